# revision 43
# baseline (speedup 1.0000x reference)
"""Distributed Trainium2 Bass kernel for BrosAttention (restructured v2).

B=2, S=1024, H=768, NH=12, DH=64:
  q,k,v = heads(hidden @ W.T + b)
  scores = q@k^T + einsum('bnid,bijd->bnij', q, bpe)   (bpe = bbox transposed)
  probs  = softmax(scores / 8)
  out    = LN(probs@v @ Wo.T + bo + hidden)

Sharding: 8 cores = 2 batches x 4 query-row blocks of 256 rows. Each core
reads only its slice of bbox_pos_emb, computes K/V for the full sequence of
its batch, writes a disjoint [256, 768] output slice. No collectives.

v2 structure (vs v1): transposed scores scoresT[j, (i)] per head; the bias
q.bpe is computed with qPair packed block-diagonally (col order 2n+s) so the
PE-transposed bias tiles are consumed RAW by the score add via strided APs
(no regroup copies). Softmax denominators come out of P@V via a 65th ones-
column on V (no ones-matmul reductions); 1/denom = exp(-ln(denom)) on ACT.
Full i=256 free dims everywhere.
"""

import os
import sys
import numpy as np

sys.path.insert(0, "/opt/trn_rl_repo")

B, S, H, NH, DH = 2, 1024, 768, 12, 64
EPS = 1e-12
P = 128
I_CORE = S * B // 8  # 256
N_CORES = 8

_COMPILED = {}

BPE_DT = "fp8"


def build_kernel(s=S, i_core=I_CORE, h=H, nh=NH, dh=DH):
    from contextlib import ExitStack
    from concourse import bacc, bass, mybir, tile

    f32 = mybir.dt.float32
    bf16 = mybir.dt.bfloat16
    fp8 = mybir.dt.float8e4
    bpe_dt = fp8 if BPE_DT == "fp8" else bf16
    Alu = mybir.AluOpType
    Act = mybir.ActivationFunctionType
    AxisX = mybir.AxisListType.X

    HC = h // P            # 6 hidden chunks
    SC = s // P            # 8 seq chunks (j)
    NPAIR = i_core // 2    # 128 i-pairs
    NOCT = i_core // 8     # 32 octos
    NOG = NOCT // 2        # 16 og-groups (2 octos = 16 i's each)
    NG = nh // 2           # 6 head pairs
    VH = h // 2            # 384

    nc = bacc.Bacc(None, target_bir_lowering=False, debug=False)

    # Steer the ACT table-load pass to the one set holding BOTH exp and ln
    # ("natural_log_exp_and_others"), so Exp/Ln alternation doesn't thrash
    # table loads. Indices stay aligned with act_info.json.
    from concourse import hw_specs
    try:
        tabs = hw_specs.get_activation_tables(nc.m.arch)
        if "natural_log_exp_and_others" in tabs:
            for name, funcs in tabs.items():
                if name != "natural_log_exp_and_others":
                    funcs.discard(mybir.ActivationFunctionType.Exp)
                    funcs.discard(mybir.ActivationFunctionType.Ln)
    except Exception:
        pass

    d_hidT = nc.declare_dram_parameter("hidT", [HC, P, s], bf16, isOutput=False)
    d_hidRT = nc.declare_dram_parameter("hidRT", [HC, P, i_core], bf16, isOutput=False)
    d_hidR = nc.declare_dram_parameter("hid_rows", [i_core // P, P, h], f32,
                                       isOutput=False)
    d_bpe = nc.declare_dram_parameter("bpe", [i_core, dh, s], bpe_dt, isOutput=False)
    d_W = {w: nc.declare_dram_parameter(w + "T", [HC, P, h], bf16, isOutput=False)
           for w in ("Wq", "Wk", "Wv", "Wo")}
    d_b = {bn: nc.declare_dram_parameter(bn, [1, h], f32, isOutput=False)
           for bn in ("bq", "bk", "bv", "bo", "ln_gamma", "ln_beta")}
    d_out = nc.declare_dram_parameter("out", [i_core // P, P, h], f32, isOutput=True)

    with tile.TileContext(nc) as tc, ExitStack() as ctx:
        # ---------------- pools ----------------
        const_p = ctx.enter_context(tc.tile_pool(name="const", bufs=1))
        stat_p = ctx.enter_context(tc.tile_pool(name="stat", bufs=1))
        # psum: psQ "qk" [P,512]f32 (one bank) x6 bufs + pctx [65,4,256] x1 = 16KB
        psQ = ctx.enter_context(
            tc.tile_pool(name="psQ", bufs=6, space=bass.MemorySpace.PSUM))
        psC = ctx.enter_context(
            tc.tile_pool(name="psC", bufs=1, space=bass.MemorySpace.PSUM))

        def big():
            return psQ.tile([P, 512], f32, name="qk")
        bpe_p = ctx.enter_context(tc.tile_pool(name="bpe", bufs=2))
        b4_p = ctx.enter_context(tc.tile_pool(name="b4", bufs=2))
        sE_p = ctx.enter_context(tc.tile_pool(name="sE", bufs=2))
        pr_p = ctx.enter_context(tc.tile_pool(name="pr", bufs=2))
        y_p = ctx.enter_context(tc.tile_pool(name="y", bufs=1))

        # ---------------- constants ----------------
        onesP = const_p.tile([P, P], bf16)
        nc.vector.memset(onesP[:], 1.0)
        ones_row = const_p.tile([1, s], bf16)
        nc.vector.memset(ones_row[:], 1.0)
        eps_t = const_p.tile([P, 1], f32)
        nc.vector.memset(eps_t[:], EPS)
        b_sb = {}
        b_bf = {}
        for bn in ("bq", "bk", "bv", "bo", "ln_gamma", "ln_beta"):
            b_sb[bn] = const_p.tile([1, h], f32, name=f"bias_{bn}")
            nc.sync.dma_start(b_sb[bn][:], d_b[bn][:])
            b_bf[bn] = const_p.tile([1, h], bf16, name=f"biasbf_{bn}")
            nc.vector.tensor_copy(b_bf[bn][:], b_sb[bn][:])

        bcast = {}
        for bn in ("ln_gamma", "ln_beta"):
            t = stat_p.tile([P, h], bf16, name=f"bcast_{bn}")
            for c in range(HC):
                pbx = big()
                nc.tensor.matmul(pbx[:, 0:P], onesP[0:1, :],
                                 b_bf[bn][:, c * P:(c + 1) * P])
                nc.scalar.copy(t[:, c * P:(c + 1) * P], pbx[:, 0:P])
            bcast[bn] = t

        # ---------------- persistent activations ----------------
        hidR = stat_p.tile([P, i_core // P, h], f32)
        nc.sync.dma_start(hidR[:], d_hidR[:].transpose([1, 0, 2]))
        WoT = stat_p.tile([P, HC, h], bf16)
        nc.sync.dma_start(WoT[:], d_W["Wo"][:].transpose([1, 0, 2]))
        qT128 = stat_p.tile([P, nh, i_core], bf16)   # q/8 duplicated both halves
        qPair = stat_p.tile([P, NPAIR, 32], bpe_dt)  # block-diag, col = 2n+s
        kT128 = stat_p.tile([P, NG, s], bf16)
        v_sb = stat_p.tile([P, SC, nh, dh + 1], bf16)  # col dh = ones
        biasT = stat_p.tile([P, NOCT, SC, P], bf16)  # raw transposed bias
        ctxT = stat_p.tile([P, NG, i_core], bf16)

        # ------- phase E (projections) interleaved with bias generation -------
        with tc.tile_pool(name="early", bufs=1) as early_p, \
             tc.tile_pool(name="earlyW", bufs=1) as earlyW_p:
            hidT = early_p.tile([P, HC, s], bf16)
            nc.sync.dma_start(hidT[:], d_hidT[:].transpose([1, 0, 2]))
            hidRT = early_p.tile([P, HC, i_core], bf16)
            nc.sync.dma_start(hidRT[:], d_hidRT[:].transpose([1, 0, 2]))

            def load_WT(w):
                t = earlyW_p.tile([P, HC, h], bf16, name="WT")
                nc.sync.dma_start(t[:], d_W[w][:].transpose([1, 0, 2]))
                return t

            # Q projection (transposed): qT = (Wq @ hidR^T + bq)/8, dup halves.
            # The 1/8 softmax scale is folded into q (QK and bias inherit it).
            WqT = load_WT("Wq")
            for r in range(HC):
                pqt = big()
                pq = pqt[:, 0:i_core]
                for kc in range(HC):
                    nc.tensor.matmul(pq, WqT[:, kc, r * P:(r + 1) * P],
                                     hidRT[:, kc, :], start=(kc == 0), stop=False)
                nc.tensor.matmul(pq, b_bf["bq"][:, r * P:(r + 1) * P],
                                 ones_row[:, 0:i_core], start=False, stop=True)
                for sub in range(2):
                    src = pqt[sub * dh:(sub + 1) * dh, 0:i_core]
                    nc.vector.tensor_scalar(qT128[0:dh, 2 * r + sub, :], src,
                                            0.125, None, Alu.mult)
                    nc.vector.tensor_scalar(qT128[dh:P, 2 * r + sub, :], src,
                                            0.125, None, Alu.mult)

            # qPair block-diag: rows 0:64 <- q even-i at cols 2n, rows 64:128 <-
            # q odd-i at cols 2n+1.
            nc.vector.memset(qPair[:], 0.0)
            nc.vector.tensor_copy(
                qPair[0:dh, :, 0:2 * nh:2],
                qT128[0:dh, :, 0::2].transpose([0, 2, 1]))
            nc.vector.tensor_copy(
                qPair[dh:P, :, 1:2 * nh:2],
                qT128[dh:P, :, 1::2].transpose([0, 2, 1]))
            nc.vector.memset(v_sb[:, :, :, dh:dh + 1], 1.0)

            WkT = load_WT("Wk")
            WvT = load_WT("Wv")

            def k_unit(r, jh):
                pkt = big()
                pk = pkt[:]
                for kc in range(HC):
                    nc.tensor.matmul(pk, WkT[:, kc, r * P:(r + 1) * P],
                                     hidT[:, kc, jh * (s // 2):(jh + 1) * (s // 2)],
                                     start=(kc == 0), stop=False)
                nc.tensor.matmul(pk, b_bf["bk"][:, r * P:(r + 1) * P],
                                 ones_row[:, 0:s // 2], start=False, stop=True)
                nc.scalar.copy(kT128[:, r, jh * (s // 2):(jh + 1) * (s // 2)],
                               pk)

            def v_unit(jc, vh):
                pvt = big()
                pv = pvt[:, 0:VH]
                for kc in range(HC):
                    nc.tensor.matmul(pv,
                                     hidT[:, kc, jc * P:(jc + 1) * P],
                                     WvT[:, kc, vh * VH:(vh + 1) * VH],
                                     start=(kc == 0), stop=False)
                nc.tensor.matmul(pv, ones_row[:, 0:P],
                                 b_bf["bv"][:, vh * VH:(vh + 1) * VH],
                                 start=False, stop=True)
                nc.scalar.copy(v_sb[:, jc, 6 * vh:6 * vh + 6, 0:dh], pv)

            def octo_unit(octo):
                # bias[n,i,j] = q[n,i,:].bpe[i,j,:] into rows 32*c4 + (2n+s),
                # j streaming; PE-transposed per j-chunk; stored RAW (dense).
                i0 = octo * 8
                bpeT = bpe_p.tile([P, 4, s], bpe_dt)
                nc.sync.dma_start(
                    bpeT[:],
                    d_bpe[i0:i0 + 8].rearrange("(a b) d j -> (b d) a j", a=4))
                pb_h = [big() for _ in range(2)]
                for c4 in range(4):
                    lhs = qPair[:, octo * 4 + c4, :]
                    for jh in range(2):
                        nc.tensor.matmul(
                            pb_h[jh][32 * c4:32 * c4 + 32, :], lhs,
                            bpeT[:, c4, jh * (s // 2):(jh + 1) * (s // 2)],
                            tile_position=(0, 32 * c4))
                b4 = b4_p.tile([P, s], bf16)
                nc.scalar.copy(b4[:, 0:s // 2], pb_h[0][:])
                nc.vector.tensor_copy(b4[:, s // 2:s], pb_h[1][:])
                nc.sync.dma_start_transpose(biasT[:, octo, :, :], b4[:])

            proj_units = ([lambda r=r, jh=jh: k_unit(r, jh)
                           for r in range(HC) for jh in range(2)] +
                          [lambda jc=jc, vh=vh: v_unit(jc, vh)
                           for jc in range(SC) for vh in range(2)])
            pi = 0
            for octo in range(NOCT):
                octo_unit(octo)
                while pi * NOCT < (octo + 1) * len(proj_units):
                    proj_units[pi]()
                    pi += 1

        # ------- attention: 4-head groups, one-bank score tiles -------
        for g4 in range(nh // 4):
            pctx = psC.tile([dh + 1, 4, i_core], f32, name="pctx")
            for jc in range(SC):
                pqk_h = [big() for _ in range(4)]
                for hn in range(4):
                    n = 4 * g4 + hn
                    bb = dh * (n & 1)
                    nc.tensor.matmul(pqk_h[hn][:, 0:i_core],
                                     kT128[bb:bb + dh, n // 2, jc * P:(jc + 1) * P],
                                     qT128[bb:bb + dh, n, :])
                sE = sE_p.tile([P, 4, i_core], bf16)
                for hn in range(4):
                    n = 4 * g4 + hn
                    nc.vector.tensor_tensor(
                        sE[:, hn, :].rearrange("p (o c u) -> p o c u", o=NOCT, c=4),
                        pqk_h[hn][:, 0:i_core].rearrange(
                            "p (o c u) -> p o c u", o=NOCT, c=4),
                        biasT[:, :, jc, :].rearrange(
                            "p o (c u) -> p o c u", c=4)[:, :, :, 2 * n:2 * n + 2],
                        Alu.add)
                probsT = pr_p.tile([P, 4, i_core], bf16)
                nc.scalar.activation(probsT[:], sE[:], Act.Exp)
                for hn in range(4):
                    n = 4 * g4 + hn
                    nc.tensor.matmul(pctx[:, hn, :], v_sb[:, jc, n, :],
                                     probsT[:, hn, :],
                                     start=(jc == 0), stop=(jc == SC - 1),
                                     skip_group_check=True)
            # evacuate ctx + denominators; 1/denom = exp(-ln(denom)) broadcast
            # to 128 partitions via K=1 matmuls, pipelined with the next group.
            denomS = y_p.tile([1, 4, i_core], bf16, name="dnm")
            for hn in range(4):
                n = 4 * g4 + hn
                r0 = dh * (n & 1)
                nc.scalar.copy(ctxT[r0:r0 + dh, n // 2, :], pctx[0:dh, hn, :])
                nc.scalar.copy(denomS[:, hn, :], pctx[dh:dh + 1, hn, :])
            prec = big()
            nc.tensor.matmul(prec[:], onesP[0:1, :],
                             denomS[:, 0:2, :].rearrange("p a b -> p (a b)"))
            prec2 = big()
            nc.tensor.matmul(prec2[:], onesP[0:1, :],
                             denomS[:, 2:4, :].rearrange("p a b -> p (a b)"))
            lgr = y_p.tile([P, i_core * 2], bf16, name="lgrA")
            nc.scalar.activation(lgr[:], prec[:], Act.Ln)
            lgr2 = y_p.tile([P, i_core * 2], bf16, name="lgrB")
            nc.scalar.activation(lgr2[:], prec2[:], Act.Ln)
            for gh, lg in ((0, lgr), (1, lgr2)):
                g = 2 * g4 + gh
                recB = y_p.tile([P, 2, i_core], bf16, name=f"recB{g % 2}")
                nc.scalar.activation(recB[:].rearrange("p a b -> p (a b)"), lg[:],
                                     Act.Exp, scale=-1.0)
                nc.vector.tensor_tensor(ctxT[0:dh, g, :], ctxT[0:dh, g, :],
                                        recB[0:dh, 0, :], Alu.mult)
                nc.vector.tensor_tensor(ctxT[dh:P, g, :], ctxT[dh:P, g, :],
                                        recB[dh:P, 1, :], Alu.mult)

        # ---------------- O-proj + residual + LN ----------------
        for half in range(2):
            i0 = half * P
            pys = [big() for _ in range(2)]
            for vh in range(2):
                for kc in range(HC):
                    nc.tensor.matmul(pys[vh][:, 0:VH], ctxT[:, kc, i0:i0 + P],
                                     WoT[:, kc, vh * VH:(vh + 1) * VH],
                                     start=(kc == 0), stop=False)
                nc.tensor.matmul(pys[vh][:, 0:VH], ones_row[:, 0:P],
                                 b_bf["bo"][:, vh * VH:(vh + 1) * VH],
                                 start=False, stop=True)
            y = y_p.tile([P, h], f32)
            for vh in range(2):
                nc.vector.tensor_tensor(y[:, vh * VH:(vh + 1) * VH],
                                        pys[vh][:, 0:VH],
                                        hidR[:, half, vh * VH:(vh + 1) * VH],
                                        Alu.add)
            mu = y_p.tile([P, 1], f32)
            nc.vector.tensor_reduce(mu[:], y[:], AxisX, Alu.add)
            nc.vector.tensor_scalar(mu[:], mu[:], 1.0 / h, None, Alu.mult)
            yc = y_p.tile([P, h], f32)
            nc.vector.tensor_scalar(yc[:], y[:], mu[:], None, Alu.subtract)
            ssq = y_p.tile([P, 1], f32)
            nc.scalar.activation(y[:], yc[:], Act.Square, accum_out=ssq[:])
            std = y_p.tile([P, 1], f32)
            nc.scalar.activation(std[:], ssq[:], Act.Sqrt,
                                 scale=1.0 / h, bias=eps_t[:])
            rstd = y_p.tile([P, 1], f32)
            nc.vector.reciprocal(rstd[:], std[:])
            o1 = y_p.tile([P, h], f32)
            nc.vector.scalar_tensor_tensor(o1[:], yc[:], rstd[:],
                                           bcast["ln_gamma"][:],
                                           Alu.mult, Alu.mult)
            nc.vector.tensor_tensor(o1[:], o1[:], bcast["ln_beta"][:], Alu.add)
            nc.sync.dma_start(d_out[half], o1[:])

    nc.compile()
    return nc


def _shard_inputs(inputs):
    import ml_dtypes
    bf = ml_dtypes.bfloat16
    bpe_np_dt = ml_dtypes.float8_e4m3 if BPE_DT == "fp8" else bf
    hs = np.ascontiguousarray(np.asarray(inputs["hidden_states"]), dtype=np.float32)
    bpe = np.asarray(inputs["bbox_pos_emb"])
    hsT = {b: np.ascontiguousarray(hs[b].T.astype(bf)).reshape(H // P, P, S)
           for b in range(B)}
    WT = {w: np.ascontiguousarray(
             np.asarray(inputs[w], dtype=np.float32).T.astype(bf)).reshape(
                 H // P, P, H)
          for w in ("Wq", "Wk", "Wv", "Wo")}
    in_maps = []
    for c in range(N_CORES):
        b = c // 4
        q0 = (c % 4) * I_CORE
        m = {
            "hidT": hsT[b],
            "hidRT": np.ascontiguousarray(
                hs[b, q0:q0 + I_CORE].T.astype(bf)).reshape(H // P, P, I_CORE),
            "hid_rows": np.ascontiguousarray(
                hs[b, q0:q0 + I_CORE].reshape(I_CORE // P, P, H)),
            "bpe": np.ascontiguousarray(
                bpe[q0:q0 + I_CORE, :, b, :].transpose(0, 2, 1).astype(bpe_np_dt)),
        }
        for w in ("Wq", "Wk", "Wv", "Wo"):
            m[w + "T"] = WT[w]
        for bn in ("bq", "bk", "bv", "bo", "ln_gamma", "ln_beta"):
            m[bn] = np.ascontiguousarray(
                np.asarray(inputs[bn], dtype=np.float32).reshape(1, H))
        in_maps.append(m)
    return in_maps


def _install_ntff_shim():
    """The agent image's antenv lacks axon_hooks; recreate the NTFF profile
    hook via ctypes against libaxon_pjrt.so so trace=True yields
    exec_time_ns + a perfetto trace."""
    import sys as _sys
    if "antenv.axon_hooks" in _sys.modules:
        return
    import types, ctypes, contextlib
    so_path = "/opt/axon/libaxon_pjrt.so"
    mod = types.ModuleType("antenv.axon_hooks")
    _state = {}

    def get_axon_ntff_profile_hook():
        if "hook" in _state:
            return _state["hook"]
        try:
            lib = ctypes.CDLL(so_path)
            if not hasattr(lib, "axon_start_nrt_profile"):
                _state["hook"] = None
                return None
            lib.axon_start_nrt_profile.argtypes = [
                ctypes.POINTER(ctypes.c_int64), ctypes.c_size_t]
            lib.axon_start_nrt_profile.restype = ctypes.c_int64
            lib.axon_stop_nrt_profile.argtypes = [ctypes.c_char_p]
            lib.axon_stop_nrt_profile.restype = ctypes.c_int64
        except OSError:
            _state["hook"] = None
            return None

        @contextlib.contextmanager
        def _hook(output_dir, device_ids):
            import jax
            jax.devices()
            if device_ids:
                ids = (ctypes.c_int64 * len(device_ids))(*device_ids)
                rc = lib.axon_start_nrt_profile(ids, len(device_ids))
            else:
                rc = lib.axon_start_nrt_profile(None, 0)
            if rc != 0:
                raise RuntimeError(f"axon_start_nrt_profile rc={rc}")
            try:
                yield
            finally:
                n = lib.axon_stop_nrt_profile(str(output_dir).encode())
                print(f"ntff profile: {n} file(s) written to {output_dir}")

        _state["hook"] = _hook
        return _hook

    mod.get_axon_ntff_profile_hook = get_axon_ntff_profile_hook
    _sys.modules["antenv.axon_hooks"] = mod


def kernel(**inputs):
    from concourse.bass_utils import run_bass_kernel_spmd

    if os.environ.get("BASS_KERNEL_TRACE"):
        _install_ntff_shim()
        import concourse.bass_utils as _bu
        _bu.upload_artifacts = lambda tmpdir: f"file://{tmpdir}"

    if "nc" not in _COMPILED:
        _COMPILED["nc"] = build_kernel()
    nc = _COMPILED["nc"]
    in_maps = _shard_inputs(inputs)
    res = run_bass_kernel_spmd(nc, in_maps, core_ids=list(range(N_CORES)),
                               trace=bool(os.environ.get("BASS_KERNEL_TRACE")))
    _COMPILED["last_result"] = res
    out = np.zeros((B, S, H), dtype=np.float32)
    for c in range(N_CORES):
        b = c // 4
        q0 = (c % 4) * I_CORE
        out[b, q0:q0 + I_CORE] = np.asarray(
            res.results[c]["out"]).reshape(I_CORE, H)
    return out


# revision 44
# speedup vs baseline: 1.4504x; 1.4504x over previous
"""Distributed Trainium2 Bass kernel for BrosAttention (restructured v2).

B=2, S=1024, H=768, NH=12, DH=64:
  q,k,v = heads(hidden @ W.T + b)
  scores = q@k^T + einsum('bnid,bijd->bnij', q, bpe)   (bpe = bbox transposed)
  probs  = softmax(scores / 8)
  out    = LN(probs@v @ Wo.T + bo + hidden)

Sharding: 8 cores = 2 batches x 4 query-row blocks of 256 rows. Each core
reads only its slice of bbox_pos_emb, computes K/V for the full sequence of
its batch, writes a disjoint [256, 768] output slice. No collectives.

v2 structure (vs v1): transposed scores scoresT[j, (i)] per head; the bias
q.bpe is computed with qPair packed block-diagonally (col order 2n+s) so the
PE-transposed bias tiles are consumed RAW by the score add via strided APs
(no regroup copies). Softmax denominators come out of P@V via a 65th ones-
column on V (no ones-matmul reductions); 1/denom = exp(-ln(denom)) on ACT.
Full i=256 free dims everywhere.
"""

import os
import sys
import numpy as np

sys.path.insert(0, "/opt/trn_rl_repo")

B, S, H, NH, DH = 2, 1024, 768, 12, 64
EPS = 1e-12
P = 128
I_CORE = S * B // 8  # 256
N_CORES = 8

_COMPILED = {}

BPE_DT = "fp8"


def build_kernel(s=S, i_core=I_CORE, h=H, nh=NH, dh=DH):
    from contextlib import ExitStack
    from concourse import bacc, bass, mybir, tile

    f32 = mybir.dt.float32
    bf16 = mybir.dt.bfloat16
    fp8 = mybir.dt.float8e4
    bpe_dt = fp8 if BPE_DT == "fp8" else bf16
    Alu = mybir.AluOpType
    Act = mybir.ActivationFunctionType
    AxisX = mybir.AxisListType.X

    HC = h // P            # 6 hidden chunks
    SC = s // P            # 8 seq chunks (j)
    NPAIR = i_core // 2    # 128 i-pairs
    NOCT = i_core // 8     # 32 octos
    NOG = NOCT // 2        # 16 og-groups (2 octos = 16 i's each)
    NG = nh // 2           # 6 head pairs
    VH = h // 2            # 384

    nc = bacc.Bacc(None, target_bir_lowering=False, debug=False)

    # Steer the ACT table-load pass to the one set holding BOTH exp and ln
    # ("natural_log_exp_and_others"), so Exp/Ln alternation doesn't thrash
    # table loads. Indices stay aligned with act_info.json.
    from concourse import hw_specs
    try:
        tabs = hw_specs.get_activation_tables(nc.m.arch)
        if "natural_log_exp_and_others" in tabs:
            for name, funcs in tabs.items():
                if name != "natural_log_exp_and_others":
                    funcs.discard(mybir.ActivationFunctionType.Exp)
                    funcs.discard(mybir.ActivationFunctionType.Ln)
    except Exception:
        pass

    d_hidT = nc.declare_dram_parameter("hidT", [HC, P, s], bf16, isOutput=False)
    d_hidRT = nc.declare_dram_parameter("hidRT", [HC, P, i_core], bf16, isOutput=False)
    d_hidR = nc.declare_dram_parameter("hid_rows", [i_core // P, P, h], f32,
                                       isOutput=False)
    d_bpe = nc.declare_dram_parameter("bpe", [i_core, dh, s], bpe_dt, isOutput=False)
    d_W = {w: nc.declare_dram_parameter(w + "T", [HC, P, h], bf16, isOutput=False)
           for w in ("Wq", "Wk", "Wv", "Wo")}
    d_b = {bn: nc.declare_dram_parameter(bn, [1, h], f32, isOutput=False)
           for bn in ("bq", "bk", "bv", "bo", "ln_gamma", "ln_beta")}
    d_ident = nc.declare_dram_parameter("ident", [P, P], bf16, isOutput=False)
    d_out = nc.declare_dram_parameter("out", [i_core // P, P, h], f32, isOutput=True)

    with tile.TileContext(nc) as tc, ExitStack() as ctx:
        # ---------------- pools ----------------
        const_p = ctx.enter_context(tc.tile_pool(name="const", bufs=1))
        stat_p = ctx.enter_context(tc.tile_pool(name="stat", bufs=1))
        # psum: psQ "qk" [P,512]f32 (one bank) x6 bufs + pctx [65,4,256] x1 = 16KB
        psQ = ctx.enter_context(
            tc.tile_pool(name="psQ", bufs=6, space=bass.MemorySpace.PSUM))
        psC = ctx.enter_context(
            tc.tile_pool(name="psC", bufs=1, space=bass.MemorySpace.PSUM))

        def big():
            return psQ.tile([P, 512], f32, name="qk")
        bpe_p = ctx.enter_context(tc.tile_pool(name="bpe", bufs=2))
        b4_p = ctx.enter_context(tc.tile_pool(name="b4", bufs=2))
        sE_p = ctx.enter_context(tc.tile_pool(name="sE", bufs=2))
        pr_p = ctx.enter_context(tc.tile_pool(name="pr", bufs=2))
        y_p = ctx.enter_context(tc.tile_pool(name="y", bufs=1))

        # ---------------- constants ----------------
        ident_bf = const_p.tile([P, P], bf16)
        nc.sync.dma_start(ident_bf[:], d_ident[:])
        onesP = const_p.tile([P, P], bf16)
        nc.vector.memset(onesP[:], 1.0)
        ones_row = const_p.tile([1, s], bf16)
        nc.vector.memset(ones_row[:], 1.0)
        eps_t = const_p.tile([P, 1], f32)
        nc.vector.memset(eps_t[:], EPS)
        b_sb = {}
        b_bf = {}
        for bn in ("bq", "bk", "bv", "bo", "ln_gamma", "ln_beta"):
            b_sb[bn] = const_p.tile([1, h], f32, name=f"bias_{bn}")
            nc.sync.dma_start(b_sb[bn][:], d_b[bn][:])
            b_bf[bn] = const_p.tile([1, h], bf16, name=f"biasbf_{bn}")
            nc.vector.tensor_copy(b_bf[bn][:], b_sb[bn][:])

        bcast = {}
        for bn in ("ln_gamma", "ln_beta"):
            t = stat_p.tile([P, h], bf16, name=f"bcast_{bn}")
            for c in range(HC):
                pbx = big()
                nc.tensor.matmul(pbx[:, 0:P], onesP[0:1, :],
                                 b_bf[bn][:, c * P:(c + 1) * P])
                nc.scalar.copy(t[:, c * P:(c + 1) * P], pbx[:, 0:P])
            bcast[bn] = t

        # ---------------- persistent activations ----------------
        hidR = stat_p.tile([P, i_core // P, h], f32)
        nc.sync.dma_start(hidR[:], d_hidR[:].transpose([1, 0, 2]))
        WoT = stat_p.tile([P, HC, h], bf16)
        nc.sync.dma_start(WoT[:], d_W["Wo"][:].transpose([1, 0, 2]))
        qT128 = stat_p.tile([P, nh, i_core], bf16)   # q/8 duplicated both halves
        qPair = stat_p.tile([P, NPAIR, 32], bpe_dt)  # block-diag, col = 2n+s
        kT128 = stat_p.tile([P, NG, s], bf16)
        v_sb = stat_p.tile([P, SC, nh, dh + 1], bf16)  # col dh = ones
        biasT = stat_p.tile([P, NOCT, SC, P], bf16)  # raw transposed bias
        ctxT = stat_p.tile([P, NG, i_core], bf16)

        # ------- phase E (projections) interleaved with bias generation -------
        with tc.tile_pool(name="early", bufs=1) as early_p, \
             tc.tile_pool(name="earlyW", bufs=1) as earlyW_p:
            hidT = early_p.tile([P, HC, s], bf16)
            nc.sync.dma_start(hidT[:], d_hidT[:].transpose([1, 0, 2]))
            hidRT = early_p.tile([P, HC, i_core], bf16)
            nc.sync.dma_start(hidRT[:], d_hidRT[:].transpose([1, 0, 2]))

            def load_WT(w):
                t = earlyW_p.tile([P, HC, h], bf16, name="WT")
                nc.sync.dma_start(t[:], d_W[w][:].transpose([1, 0, 2]))
                return t

            # Q projection (transposed): qT = (Wq @ hidR^T + bq)/8, dup halves.
            # The 1/8 softmax scale is folded into q (QK and bias inherit it).
            WqT = load_WT("Wq")
            for r in range(HC):
                pqt = big()
                pq = pqt[:, 0:i_core]
                for kc in range(HC):
                    nc.tensor.matmul(pq, WqT[:, kc, r * P:(r + 1) * P],
                                     hidRT[:, kc, :], start=(kc == 0), stop=False)
                nc.tensor.matmul(pq, b_bf["bq"][:, r * P:(r + 1) * P],
                                 ones_row[:, 0:i_core], start=False, stop=True)
                for sub in range(2):
                    src = pqt[sub * dh:(sub + 1) * dh, 0:i_core]
                    nc.vector.tensor_scalar(qT128[0:dh, 2 * r + sub, :], src,
                                            0.125, None, Alu.mult)
                    nc.vector.tensor_scalar(qT128[dh:P, 2 * r + sub, :], src,
                                            0.125, None, Alu.mult)

            # qPair block-diag: rows 0:64 <- q even-i at cols 2n, rows 64:128 <-
            # q odd-i at cols 2n+1.
            nc.vector.memset(qPair[:], 0.0)
            nc.vector.tensor_copy(
                qPair[0:dh, :, 0:2 * nh:2],
                qT128[0:dh, :, 0::2].transpose([0, 2, 1]))
            nc.vector.tensor_copy(
                qPair[dh:P, :, 1:2 * nh:2],
                qT128[dh:P, :, 1::2].transpose([0, 2, 1]))
            nc.vector.memset(v_sb[:, :, :, dh:dh + 1], 1.0)

            WkT = load_WT("Wk")
            WvT = load_WT("Wv")

            def k_unit(r, jh):
                pkt = big()
                pk = pkt[:]
                for kc in range(HC):
                    nc.tensor.matmul(pk, WkT[:, kc, r * P:(r + 1) * P],
                                     hidT[:, kc, jh * (s // 2):(jh + 1) * (s // 2)],
                                     start=(kc == 0), stop=False)
                nc.tensor.matmul(pk, b_bf["bk"][:, r * P:(r + 1) * P],
                                 ones_row[:, 0:s // 2], start=False, stop=True)
                nc.scalar.copy(kT128[:, r, jh * (s // 2):(jh + 1) * (s // 2)],
                               pk)

            def v_unit(jc, vh):
                pvt = big()
                pv = pvt[:, 0:VH]
                for kc in range(HC):
                    nc.tensor.matmul(pv,
                                     hidT[:, kc, jc * P:(jc + 1) * P],
                                     WvT[:, kc, vh * VH:(vh + 1) * VH],
                                     start=(kc == 0), stop=False)
                nc.tensor.matmul(pv, ones_row[:, 0:P],
                                 b_bf["bv"][:, vh * VH:(vh + 1) * VH],
                                 start=False, stop=True)
                nc.scalar.copy(v_sb[:, jc, 6 * vh:6 * vh + 6, 0:dh], pv)

            def octo_unit(octo):
                # bias[n,i,j] = q[n,i,:].bpe[i,j,:] into rows 32*c4 + (2n+s),
                # j streaming; PE-transposed per j-chunk; stored RAW (dense).
                i0 = octo * 8
                bpeT = bpe_p.tile([P, 4, s], bpe_dt)
                nc.sync.dma_start(
                    bpeT[:],
                    d_bpe[i0:i0 + 8].rearrange("(a b) d j -> (b d) a j", a=4))
                pb_h = [big() for _ in range(2)]
                for c4 in range(4):
                    lhs = qPair[:, octo * 4 + c4, :]
                    for jh in range(2):
                        nc.tensor.matmul(
                            pb_h[jh][32 * c4:32 * c4 + 32, :], lhs,
                            bpeT[:, c4, jh * (s // 2):(jh + 1) * (s // 2)],
                            tile_position=(0, 32 * c4))
                b4 = b4_p.tile([P, s], bf16)
                nc.scalar.copy(b4[:, 0:s // 2], pb_h[0][:])
                nc.vector.tensor_copy(b4[:, s // 2:s], pb_h[1][:])
                ptb = big().bitcast(bf16).rearrange("p (j u) -> p j u", j=SC)
                for jc in range(SC):
                    nc.tensor.transpose(ptb[:, jc, :], b4[:, jc * P:(jc + 1) * P],
                                        ident_bf[:])
                if octo % 2 == 0:
                    nc.scalar.copy(biasT[:, octo, :, :], ptb)
                else:
                    nc.vector.tensor_copy(biasT[:, octo, :, :], ptb)

            proj_units = ([lambda r=r, jh=jh: k_unit(r, jh)
                           for r in range(HC) for jh in range(2)] +
                          [lambda jc=jc, vh=vh: v_unit(jc, vh)
                           for jc in range(SC) for vh in range(2)])
            pi = 0
            for octo in range(NOCT):
                octo_unit(octo)
                while pi * NOCT < (octo + 1) * len(proj_units):
                    proj_units[pi]()
                    pi += 1

        # ------- attention: 4-head groups, one-bank score tiles -------
        for g4 in range(nh // 4):
            pctx = psC.tile([dh + 1, 4, i_core], f32, name="pctx")
            for jc in range(SC):
                pqk_h = [big() for _ in range(4)]
                for hn in range(4):
                    n = 4 * g4 + hn
                    bb = dh * (n & 1)
                    nc.tensor.matmul(pqk_h[hn][:, 0:i_core],
                                     kT128[bb:bb + dh, n // 2, jc * P:(jc + 1) * P],
                                     qT128[bb:bb + dh, n, :])
                sE = sE_p.tile([P, 4, i_core], bf16)
                for hn in range(4):
                    n = 4 * g4 + hn
                    nc.vector.tensor_tensor(
                        sE[:, hn, :].rearrange("p (o c u) -> p o c u", o=NOCT, c=4),
                        pqk_h[hn][:, 0:i_core].rearrange(
                            "p (o c u) -> p o c u", o=NOCT, c=4),
                        biasT[:, :, jc, :].rearrange(
                            "p o (c u) -> p o c u", c=4)[:, :, :, 2 * n:2 * n + 2],
                        Alu.add)
                probsT = pr_p.tile([P, 4, i_core], bf16)
                nc.scalar.activation(probsT[:], sE[:], Act.Exp)
                for hn in range(4):
                    n = 4 * g4 + hn
                    nc.tensor.matmul(pctx[:, hn, :], v_sb[:, jc, n, :],
                                     probsT[:, hn, :],
                                     start=(jc == 0), stop=(jc == SC - 1),
                                     skip_group_check=True)
            # evacuate ctx + denominators; 1/denom = exp(-ln(denom)) broadcast
            # to 128 partitions via K=1 matmuls, pipelined with the next group.
            denomS = y_p.tile([1, 4, i_core], bf16, name="dnm")
            for hn in range(4):
                n = 4 * g4 + hn
                r0 = dh * (n & 1)
                nc.scalar.copy(ctxT[r0:r0 + dh, n // 2, :], pctx[0:dh, hn, :])
                nc.scalar.copy(denomS[:, hn, :], pctx[dh:dh + 1, hn, :])
            prec = big()
            nc.tensor.matmul(prec[:], onesP[0:1, :],
                             denomS[:, 0:2, :].rearrange("p a b -> p (a b)"))
            prec2 = big()
            nc.tensor.matmul(prec2[:], onesP[0:1, :],
                             denomS[:, 2:4, :].rearrange("p a b -> p (a b)"))
            lgr = y_p.tile([P, i_core * 2], bf16, name="lgrA")
            nc.scalar.activation(lgr[:], prec[:], Act.Ln)
            lgr2 = y_p.tile([P, i_core * 2], bf16, name="lgrB")
            nc.scalar.activation(lgr2[:], prec2[:], Act.Ln)
            for gh, lg in ((0, lgr), (1, lgr2)):
                g = 2 * g4 + gh
                recB = y_p.tile([P, 2, i_core], bf16, name=f"recB{g % 2}")
                nc.scalar.activation(recB[:].rearrange("p a b -> p (a b)"), lg[:],
                                     Act.Exp, scale=-1.0)
                nc.vector.tensor_tensor(ctxT[0:dh, g, :], ctxT[0:dh, g, :],
                                        recB[0:dh, 0, :], Alu.mult)
                nc.vector.tensor_tensor(ctxT[dh:P, g, :], ctxT[dh:P, g, :],
                                        recB[dh:P, 1, :], Alu.mult)

        # ---------------- O-proj + residual + LN ----------------
        for half in range(2):
            i0 = half * P
            pys = [big() for _ in range(2)]
            for vh in range(2):
                for kc in range(HC):
                    nc.tensor.matmul(pys[vh][:, 0:VH], ctxT[:, kc, i0:i0 + P],
                                     WoT[:, kc, vh * VH:(vh + 1) * VH],
                                     start=(kc == 0), stop=False)
                nc.tensor.matmul(pys[vh][:, 0:VH], ones_row[:, 0:P],
                                 b_bf["bo"][:, vh * VH:(vh + 1) * VH],
                                 start=False, stop=True)
            y = y_p.tile([P, h], f32)
            for vh in range(2):
                nc.vector.tensor_tensor(y[:, vh * VH:(vh + 1) * VH],
                                        pys[vh][:, 0:VH],
                                        hidR[:, half, vh * VH:(vh + 1) * VH],
                                        Alu.add)
            mu = y_p.tile([P, 1], f32)
            nc.vector.tensor_reduce(mu[:], y[:], AxisX, Alu.add)
            nc.vector.tensor_scalar(mu[:], mu[:], 1.0 / h, None, Alu.mult)
            yc = y_p.tile([P, h], f32)
            nc.vector.tensor_scalar(yc[:], y[:], mu[:], None, Alu.subtract)
            ssq = y_p.tile([P, 1], f32)
            nc.scalar.activation(y[:], yc[:], Act.Square, accum_out=ssq[:])
            std = y_p.tile([P, 1], f32)
            nc.scalar.activation(std[:], ssq[:], Act.Sqrt,
                                 scale=1.0 / h, bias=eps_t[:])
            rstd = y_p.tile([P, 1], f32)
            nc.vector.reciprocal(rstd[:], std[:])
            o1 = y_p.tile([P, h], f32)
            nc.vector.scalar_tensor_tensor(o1[:], yc[:], rstd[:],
                                           bcast["ln_gamma"][:],
                                           Alu.mult, Alu.mult)
            nc.vector.tensor_tensor(o1[:], o1[:], bcast["ln_beta"][:], Alu.add)
            nc.sync.dma_start(d_out[half], o1[:])

    nc.compile()
    return nc


def _shard_inputs(inputs):
    import ml_dtypes
    bf = ml_dtypes.bfloat16
    bpe_np_dt = ml_dtypes.float8_e4m3 if BPE_DT == "fp8" else bf
    hs = np.ascontiguousarray(np.asarray(inputs["hidden_states"]), dtype=np.float32)
    bpe = np.asarray(inputs["bbox_pos_emb"])
    ident = np.eye(P, dtype=np.float32).astype(bf)
    hsT = {b: np.ascontiguousarray(hs[b].T.astype(bf)).reshape(H // P, P, S)
           for b in range(B)}
    WT = {w: np.ascontiguousarray(
             np.asarray(inputs[w], dtype=np.float32).T.astype(bf)).reshape(
                 H // P, P, H)
          for w in ("Wq", "Wk", "Wv", "Wo")}
    in_maps = []
    for c in range(N_CORES):
        b = c // 4
        q0 = (c % 4) * I_CORE
        m = {
            "ident": ident,
            "hidT": hsT[b],
            "hidRT": np.ascontiguousarray(
                hs[b, q0:q0 + I_CORE].T.astype(bf)).reshape(H // P, P, I_CORE),
            "hid_rows": np.ascontiguousarray(
                hs[b, q0:q0 + I_CORE].reshape(I_CORE // P, P, H)),
            "bpe": np.ascontiguousarray(
                bpe[q0:q0 + I_CORE, :, b, :].transpose(0, 2, 1).astype(bpe_np_dt)),
        }
        for w in ("Wq", "Wk", "Wv", "Wo"):
            m[w + "T"] = WT[w]
        for bn in ("bq", "bk", "bv", "bo", "ln_gamma", "ln_beta"):
            m[bn] = np.ascontiguousarray(
                np.asarray(inputs[bn], dtype=np.float32).reshape(1, H))
        in_maps.append(m)
    return in_maps


def _install_ntff_shim():
    """The agent image's antenv lacks axon_hooks; recreate the NTFF profile
    hook via ctypes against libaxon_pjrt.so so trace=True yields
    exec_time_ns + a perfetto trace."""
    import sys as _sys
    if "antenv.axon_hooks" in _sys.modules:
        return
    import types, ctypes, contextlib
    so_path = "/opt/axon/libaxon_pjrt.so"
    mod = types.ModuleType("antenv.axon_hooks")
    _state = {}

    def get_axon_ntff_profile_hook():
        if "hook" in _state:
            return _state["hook"]
        try:
            lib = ctypes.CDLL(so_path)
            if not hasattr(lib, "axon_start_nrt_profile"):
                _state["hook"] = None
                return None
            lib.axon_start_nrt_profile.argtypes = [
                ctypes.POINTER(ctypes.c_int64), ctypes.c_size_t]
            lib.axon_start_nrt_profile.restype = ctypes.c_int64
            lib.axon_stop_nrt_profile.argtypes = [ctypes.c_char_p]
            lib.axon_stop_nrt_profile.restype = ctypes.c_int64
        except OSError:
            _state["hook"] = None
            return None

        @contextlib.contextmanager
        def _hook(output_dir, device_ids):
            import jax
            jax.devices()
            if device_ids:
                ids = (ctypes.c_int64 * len(device_ids))(*device_ids)
                rc = lib.axon_start_nrt_profile(ids, len(device_ids))
            else:
                rc = lib.axon_start_nrt_profile(None, 0)
            if rc != 0:
                raise RuntimeError(f"axon_start_nrt_profile rc={rc}")
            try:
                yield
            finally:
                n = lib.axon_stop_nrt_profile(str(output_dir).encode())
                print(f"ntff profile: {n} file(s) written to {output_dir}")

        _state["hook"] = _hook
        return _hook

    mod.get_axon_ntff_profile_hook = get_axon_ntff_profile_hook
    _sys.modules["antenv.axon_hooks"] = mod


def kernel(**inputs):
    from concourse.bass_utils import run_bass_kernel_spmd

    if os.environ.get("BASS_KERNEL_TRACE"):
        _install_ntff_shim()
        import concourse.bass_utils as _bu
        _bu.upload_artifacts = lambda tmpdir: f"file://{tmpdir}"

    if "nc" not in _COMPILED:
        _COMPILED["nc"] = build_kernel()
    nc = _COMPILED["nc"]
    in_maps = _shard_inputs(inputs)
    res = run_bass_kernel_spmd(nc, in_maps, core_ids=list(range(N_CORES)),
                               trace=bool(os.environ.get("BASS_KERNEL_TRACE")))
    _COMPILED["last_result"] = res
    out = np.zeros((B, S, H), dtype=np.float32)
    for c in range(N_CORES):
        b = c // 4
        q0 = (c % 4) * I_CORE
        out[b, q0:q0 + I_CORE] = np.asarray(
            res.results[c]["out"]).reshape(I_CORE, H)
    return out


# revision 47
# speedup vs baseline: 1.5077x; 1.0395x over previous
"""Distributed Trainium2 Bass kernel for BrosAttention (restructured v2).

B=2, S=1024, H=768, NH=12, DH=64:
  q,k,v = heads(hidden @ W.T + b)
  scores = q@k^T + einsum('bnid,bijd->bnij', q, bpe)   (bpe = bbox transposed)
  probs  = softmax(scores / 8)
  out    = LN(probs@v @ Wo.T + bo + hidden)

Sharding: 8 cores = 2 batches x 4 query-row blocks of 256 rows. Each core
reads only its slice of bbox_pos_emb, computes K/V for the full sequence of
its batch, writes a disjoint [256, 768] output slice. No collectives.

v2 structure (vs v1): transposed scores scoresT[j, (i)] per head; the bias
q.bpe is computed with qPair packed block-diagonally (col order 2n+s) so the
PE-transposed bias tiles are consumed RAW by the score add via strided APs
(no regroup copies). Softmax denominators come out of P@V via a 65th ones-
column on V (no ones-matmul reductions); 1/denom = exp(-ln(denom)) on ACT.
Full i=256 free dims everywhere.
"""

import os
import sys
import numpy as np

sys.path.insert(0, "/opt/trn_rl_repo")

B, S, H, NH, DH = 2, 1024, 768, 12, 64
EPS = 1e-12
P = 128
I_CORE = S * B // 8  # 256
N_CORES = 8

_COMPILED = {}

BPE_DT = "fp8"


def build_kernel(s=S, i_core=I_CORE, h=H, nh=NH, dh=DH):
    from contextlib import ExitStack
    from concourse import bacc, bass, mybir, tile

    f32 = mybir.dt.float32
    bf16 = mybir.dt.bfloat16
    fp8 = mybir.dt.float8e4
    bpe_dt = fp8 if BPE_DT == "fp8" else bf16
    Alu = mybir.AluOpType
    Act = mybir.ActivationFunctionType
    AxisX = mybir.AxisListType.X

    HC = h // P            # 6 hidden chunks
    SC = s // P            # 8 seq chunks (j)
    NPAIR = i_core // 2    # 128 i-pairs
    NOCT = i_core // 8     # 32 octos
    NOG = NOCT // 2        # 16 og-groups (2 octos = 16 i's each)
    NG = nh // 2           # 6 head pairs
    VH = h // 2            # 384

    nc = bacc.Bacc(None, target_bir_lowering=False, debug=False)

    # Steer the ACT table-load pass to the one set holding BOTH exp and ln
    # ("natural_log_exp_and_others"), so Exp/Ln alternation doesn't thrash
    # table loads. Indices stay aligned with act_info.json.
    from concourse import hw_specs
    try:
        tabs = hw_specs.get_activation_tables(nc.m.arch)
        if "natural_log_exp_and_others" in tabs:
            for name, funcs in tabs.items():
                if name != "natural_log_exp_and_others":
                    funcs.discard(mybir.ActivationFunctionType.Exp)
                    funcs.discard(mybir.ActivationFunctionType.Ln)
    except Exception:
        pass

    d_hidR = nc.declare_dram_parameter("hid_rows", [i_core // P, P, h], f32,
                                       isOutput=False)
    d_bpe = nc.declare_dram_parameter("bpe", [i_core, dh, s], bpe_dt, isOutput=False)
    d_W = {"Wo": nc.declare_dram_parameter("WoT", [HC, P, h], bf16,
                                           isOutput=False)}
    for w in ("Wq", "Wk", "Wv"):
        d_W[w] = nc.declare_dram_parameter(w + "8", [HC // 2, P, 2, h], fp8,
                                           isOutput=False)
    d_hidT8 = nc.declare_dram_parameter("hidT8", [HC, P, s], fp8, isOutput=False)
    d_hidRT8 = nc.declare_dram_parameter("hidRT8", [HC, P, i_core], fp8,
                                         isOutput=False)
    d_b = {bn: nc.declare_dram_parameter(bn, [1, h], f32, isOutput=False)
           for bn in ("bq", "bk", "bv", "bo", "ln_gamma", "ln_beta")}
    d_ident = nc.declare_dram_parameter("ident", [P, P], bf16, isOutput=False)
    d_out = nc.declare_dram_parameter("out", [i_core // P, P, h], f32, isOutput=True)

    with tile.TileContext(nc) as tc, ExitStack() as ctx:
        # ---------------- pools ----------------
        const_p = ctx.enter_context(tc.tile_pool(name="const", bufs=1))
        stat_p = ctx.enter_context(tc.tile_pool(name="stat", bufs=1))
        # psum: psQ "qk" [P,512]f32 (one bank) x6 bufs + pctx [65,4,256] x1 = 16KB
        psQ = ctx.enter_context(
            tc.tile_pool(name="psQ", bufs=6, space=bass.MemorySpace.PSUM))
        psC = ctx.enter_context(
            tc.tile_pool(name="psC", bufs=1, space=bass.MemorySpace.PSUM))

        def big():
            return psQ.tile([P, 512], f32, name="qk")
        bpe_p = ctx.enter_context(tc.tile_pool(name="bpe", bufs=2))
        b4_p = ctx.enter_context(tc.tile_pool(name="b4", bufs=2))
        sE_p = ctx.enter_context(tc.tile_pool(name="sE", bufs=2))
        pr_p = ctx.enter_context(tc.tile_pool(name="pr", bufs=2))
        y_p = ctx.enter_context(tc.tile_pool(name="y", bufs=1))

        # ---------------- constants ----------------
        ident_bf = const_p.tile([P, P], bf16)
        nc.sync.dma_start(ident_bf[:], d_ident[:])
        onesP = const_p.tile([P, P], bf16)
        nc.vector.memset(onesP[:], 1.0)
        ones_row = const_p.tile([1, s], bf16)
        nc.vector.memset(ones_row[:], 1.0)
        eps_t = const_p.tile([P, 1], f32)
        nc.vector.memset(eps_t[:], EPS)
        b_sb = {}
        b_bf = {}
        for bn in ("bq", "bk", "bv", "bo", "ln_gamma", "ln_beta"):
            b_sb[bn] = const_p.tile([1, h], f32, name=f"bias_{bn}")
            nc.sync.dma_start(b_sb[bn][:], d_b[bn][:])
            b_bf[bn] = const_p.tile([1, h], bf16, name=f"biasbf_{bn}")
            nc.vector.tensor_copy(b_bf[bn][:], b_sb[bn][:])

        bcast = {}
        for bn in ("ln_gamma", "ln_beta"):
            t = stat_p.tile([P, h], bf16, name=f"bcast_{bn}")
            for c in range(HC):
                pbx = big()
                nc.tensor.matmul(pbx[:, 0:P], onesP[0:1, :],
                                 b_bf[bn][:, c * P:(c + 1) * P])
                nc.scalar.copy(t[:, c * P:(c + 1) * P], pbx[:, 0:P])
            bcast[bn] = t

        # ---------------- persistent activations ----------------
        hidR = stat_p.tile([P, i_core // P, h], f32)
        nc.sync.dma_start(hidR[:], d_hidR[:].transpose([1, 0, 2]))
        WoT = stat_p.tile([P, HC, h], bf16)
        nc.sync.dma_start(WoT[:], d_W["Wo"][:].transpose([1, 0, 2]))
        qT128 = stat_p.tile([P, nh, i_core], bf16)   # q/8 duplicated both halves
        qPair = stat_p.tile([P, NPAIR, 32], bpe_dt)  # block-diag, col = 2n+s
        kT128 = stat_p.tile([P, NG, s], bf16)
        v_sb = stat_p.tile([P, SC, nh, dh + 1], bf16)  # col dh = ones
        biasT = stat_p.tile([P, NOCT, SC, P], bf16)  # raw transposed bias
        ctxT = stat_p.tile([P, NG, i_core], bf16)

        # ------- phase E (projections) interleaved with bias generation -------
        with tc.tile_pool(name="early", bufs=1) as early_p, \
             tc.tile_pool(name="earlyW", bufs=1) as earlyW_p:
            hidT = early_p.tile([P, HC, s], fp8)
            nc.sync.dma_start(hidT[:], d_hidT8[:].transpose([1, 0, 2]))
            hidRT = early_p.tile([P, HC, i_core], fp8)
            nc.sync.dma_start(hidRT[:], d_hidRT8[:].transpose([1, 0, 2]))

            def load_WT(w):
                t = earlyW_p.tile([P, HC // 2, 2, h], fp8, name="WT")
                nc.sync.dma_start(t[:], d_W[w][:].transpose([1, 0, 2, 3]))
                return t

            # Q projection (transposed): qT = (Wq @ hidR^T + bq)/8, dup halves.
            # The 1/8 softmax scale is folded into q (QK and bias inherit it).
            WqT = load_WT("Wq")
            DR = mybir.MatmulPerfMode.DoubleRow
            for r in range(HC):
                pqt = big()
                pq = pqt[:, 0:i_core]
                for kcp in range(HC // 2):
                    nc.tensor.matmul(
                        pq, WqT[:, kcp, :, r * P:(r + 1) * P],
                        hidRT[:, 2 * kcp:2 * kcp + 2, :],
                        start=(kcp == 0), stop=False, perf_mode=DR)
                nc.tensor.matmul(pq, b_bf["bq"][:, r * P:(r + 1) * P],
                                 ones_row[:, 0:i_core], start=False, stop=True)
                for sub in range(2):
                    src = pqt[sub * dh:(sub + 1) * dh, 0:i_core]
                    nc.vector.tensor_scalar(qT128[0:dh, 2 * r + sub, :], src,
                                            0.125, None, Alu.mult)
                    nc.vector.tensor_scalar(qT128[dh:P, 2 * r + sub, :], src,
                                            0.125, None, Alu.mult)

            # qPair block-diag: rows 0:64 <- q even-i at cols 2n, rows 64:128 <-
            # q odd-i at cols 2n+1.
            nc.vector.memset(qPair[:], 0.0)
            nc.vector.tensor_copy(
                qPair[0:dh, :, 0:2 * nh:2],
                qT128[0:dh, :, 0::2].transpose([0, 2, 1]))
            nc.vector.tensor_copy(
                qPair[dh:P, :, 1:2 * nh:2],
                qT128[dh:P, :, 1::2].transpose([0, 2, 1]))
            nc.vector.memset(v_sb[:, :, :, dh:dh + 1], 1.0)

            WkT = load_WT("Wk")
            WvT = load_WT("Wv")

            def k_unit(r, jh):
                pkt = big()
                pk = pkt[:]
                for kcp in range(HC // 2):
                    nc.tensor.matmul(
                        pk, WkT[:, kcp, :, r * P:(r + 1) * P],
                        hidT[:, 2 * kcp:2 * kcp + 2,
                             jh * (s // 2):(jh + 1) * (s // 2)],
                        start=(kcp == 0), stop=False, perf_mode=DR)
                nc.tensor.matmul(pk, b_bf["bk"][:, r * P:(r + 1) * P],
                                 ones_row[:, 0:s // 2], start=False, stop=True)
                nc.scalar.copy(kT128[:, r, jh * (s // 2):(jh + 1) * (s // 2)],
                               pk)

            def v_unit(jc, vh):
                pvt = big()
                pv = pvt[:, 0:VH]
                for kcp in range(HC // 2):
                    nc.tensor.matmul(
                        pv,
                        hidT[:, 2 * kcp:2 * kcp + 2, jc * P:(jc + 1) * P],
                        WvT[:, kcp, :, vh * VH:(vh + 1) * VH],
                        start=(kcp == 0), stop=False, perf_mode=DR)
                nc.tensor.matmul(pv, ones_row[:, 0:P],
                                 b_bf["bv"][:, vh * VH:(vh + 1) * VH],
                                 start=False, stop=True)
                nc.scalar.copy(v_sb[:, jc, 6 * vh:6 * vh + 6, 0:dh], pv)

            def octo_unit(octo):
                # bias[n,i,j] = q[n,i,:].bpe[i,j,:] into rows 32*c4 + (2n+s),
                # j streaming; PE-transposed per j-chunk; stored RAW (dense).
                i0 = octo * 8
                bpeT = bpe_p.tile([P, 4, s], bpe_dt)
                nc.sync.dma_start(
                    bpeT[:],
                    d_bpe[i0:i0 + 8].rearrange("(a b) d j -> (b d) a j", a=4))
                pb_h = [big() for _ in range(2)]
                for c4 in range(4):
                    lhs = qPair[:, octo * 4 + c4, :]
                    for jh in range(2):
                        nc.tensor.matmul(
                            pb_h[jh][32 * c4:32 * c4 + 32, :], lhs,
                            bpeT[:, c4, jh * (s // 2):(jh + 1) * (s // 2)],
                            tile_position=(0, 32 * c4))
                b4 = b4_p.tile([P, s], bf16)
                nc.scalar.copy(b4[:, 0:s // 2], pb_h[0][:])
                nc.vector.tensor_copy(b4[:, s // 2:s], pb_h[1][:])
                ptb = big().bitcast(bf16).rearrange("p (j u) -> p j u", j=SC)
                for jc in range(SC):
                    nc.tensor.transpose(ptb[:, jc, :], b4[:, jc * P:(jc + 1) * P],
                                        ident_bf[:])
                if octo % 2 == 0:
                    nc.scalar.copy(biasT[:, octo, :, :], ptb)
                else:
                    nc.vector.tensor_copy(biasT[:, octo, :, :], ptb)

            proj_units = ([lambda r=r, jh=jh: k_unit(r, jh)
                           for r in range(HC) for jh in range(2)] +
                          [lambda jc=jc, vh=vh: v_unit(jc, vh)
                           for jc in range(SC) for vh in range(2)])
            pi = 0
            for octo in range(NOCT):
                octo_unit(octo)
                while pi * NOCT < (octo + 1) * len(proj_units):
                    proj_units[pi]()
                    pi += 1

        # ------- attention: 4-head groups, one-bank score tiles -------
        for g4 in range(nh // 4):
            pctx = psC.tile([dh + 1, 4, i_core], f32, name="pctx")
            for jc in range(SC):
                pqk_h = [big() for _ in range(4)]
                for hn in range(4):
                    n = 4 * g4 + hn
                    bb = dh * (n & 1)
                    nc.tensor.matmul(pqk_h[hn][:, 0:i_core],
                                     kT128[bb:bb + dh, n // 2, jc * P:(jc + 1) * P],
                                     qT128[bb:bb + dh, n, :])
                sE = sE_p.tile([P, 4, i_core], bf16)
                for hn in range(4):
                    n = 4 * g4 + hn
                    nc.vector.tensor_tensor(
                        sE[:, hn, :].rearrange("p (o c u) -> p o c u", o=NOCT, c=4),
                        pqk_h[hn][:, 0:i_core].rearrange(
                            "p (o c u) -> p o c u", o=NOCT, c=4),
                        biasT[:, :, jc, :].rearrange(
                            "p o (c u) -> p o c u", c=4)[:, :, :, 2 * n:2 * n + 2],
                        Alu.add)
                probsT = pr_p.tile([P, 4, i_core], bf16)
                nc.scalar.activation(probsT[:], sE[:], Act.Exp)
                for hn in range(4):
                    n = 4 * g4 + hn
                    nc.tensor.matmul(pctx[:, hn, :], v_sb[:, jc, n, :],
                                     probsT[:, hn, :],
                                     start=(jc == 0), stop=(jc == SC - 1),
                                     skip_group_check=True)
            # evacuate ctx + denominators; 1/denom = exp(-ln(denom)) broadcast
            # to 128 partitions via K=1 matmuls, pipelined with the next group.
            denomS = y_p.tile([1, 4, i_core], bf16, name="dnm")
            for hn in range(4):
                n = 4 * g4 + hn
                r0 = dh * (n & 1)
                nc.scalar.copy(ctxT[r0:r0 + dh, n // 2, :], pctx[0:dh, hn, :])
                nc.scalar.copy(denomS[:, hn, :], pctx[dh:dh + 1, hn, :])
            prec = big()
            nc.tensor.matmul(prec[:], onesP[0:1, :],
                             denomS[:, 0:2, :].rearrange("p a b -> p (a b)"))
            prec2 = big()
            nc.tensor.matmul(prec2[:], onesP[0:1, :],
                             denomS[:, 2:4, :].rearrange("p a b -> p (a b)"))
            lgr = y_p.tile([P, i_core * 2], bf16, name="lgrA")
            nc.scalar.activation(lgr[:], prec[:], Act.Ln)
            lgr2 = y_p.tile([P, i_core * 2], bf16, name="lgrB")
            nc.scalar.activation(lgr2[:], prec2[:], Act.Ln)
            for gh, lg in ((0, lgr), (1, lgr2)):
                g = 2 * g4 + gh
                recB = y_p.tile([P, 2, i_core], bf16, name=f"recB{g % 2}")
                nc.scalar.activation(recB[:].rearrange("p a b -> p (a b)"), lg[:],
                                     Act.Exp, scale=-1.0)
                nc.vector.tensor_tensor(ctxT[0:dh, g, :], ctxT[0:dh, g, :],
                                        recB[0:dh, 0, :], Alu.mult)
                nc.vector.tensor_tensor(ctxT[dh:P, g, :], ctxT[dh:P, g, :],
                                        recB[dh:P, 1, :], Alu.mult)

        # ---------------- O-proj + residual + LN ----------------
        for half in range(2):
            i0 = half * P
            pys = [big() for _ in range(2)]
            for vh in range(2):
                for kc in range(HC):
                    nc.tensor.matmul(pys[vh][:, 0:VH], ctxT[:, kc, i0:i0 + P],
                                     WoT[:, kc, vh * VH:(vh + 1) * VH],
                                     start=(kc == 0), stop=False)
                nc.tensor.matmul(pys[vh][:, 0:VH], ones_row[:, 0:P],
                                 b_bf["bo"][:, vh * VH:(vh + 1) * VH],
                                 start=False, stop=True)
            y = y_p.tile([P, h], f32)
            for vh in range(2):
                nc.vector.tensor_tensor(y[:, vh * VH:(vh + 1) * VH],
                                        pys[vh][:, 0:VH],
                                        hidR[:, half, vh * VH:(vh + 1) * VH],
                                        Alu.add)
            mu = y_p.tile([P, 1], f32)
            nc.vector.tensor_reduce(mu[:], y[:], AxisX, Alu.add)
            nc.vector.tensor_scalar(mu[:], mu[:], 1.0 / h, None, Alu.mult)
            yc = y_p.tile([P, h], f32)
            nc.vector.tensor_scalar(yc[:], y[:], mu[:], None, Alu.subtract)
            ssq = y_p.tile([P, 1], f32)
            nc.scalar.activation(y[:], yc[:], Act.Square, accum_out=ssq[:])
            std = y_p.tile([P, 1], f32)
            nc.scalar.activation(std[:], ssq[:], Act.Sqrt,
                                 scale=1.0 / h, bias=eps_t[:])
            rstd = y_p.tile([P, 1], f32)
            nc.vector.reciprocal(rstd[:], std[:])
            o1 = y_p.tile([P, h], f32)
            nc.vector.scalar_tensor_tensor(o1[:], yc[:], rstd[:],
                                           bcast["ln_gamma"][:],
                                           Alu.mult, Alu.mult)
            nc.vector.tensor_tensor(o1[:], o1[:], bcast["ln_beta"][:], Alu.add)
            nc.sync.dma_start(d_out[half], o1[:])

    nc.compile()
    return nc


def _shard_inputs(inputs):
    import ml_dtypes
    bf = ml_dtypes.bfloat16
    f8 = ml_dtypes.float8_e4m3
    bpe_np_dt = f8 if BPE_DT == "fp8" else bf
    hs = np.ascontiguousarray(np.asarray(inputs["hidden_states"]), dtype=np.float32)
    bpe = np.asarray(inputs["bbox_pos_emb"])
    ident = np.eye(P, dtype=np.float32).astype(bf)
    hsT8 = {b: np.ascontiguousarray(hs[b].T.astype(f8)).reshape(H // P, P, S)
            for b in range(B)}
    WoT = np.ascontiguousarray(
        np.asarray(inputs["Wo"], dtype=np.float32).T.astype(bf)).reshape(
            H // P, P, H)
    W8 = {w: np.ascontiguousarray(
             np.asarray(inputs[w], dtype=np.float32).T.astype(f8).reshape(
                 H // 256, 2, P, H).transpose(0, 2, 1, 3))
          for w in ("Wq", "Wk", "Wv")}
    in_maps = []
    for c in range(N_CORES):
        b = c // 4
        q0 = (c % 4) * I_CORE
        m = {
            "ident": ident,
            "hidT8": hsT8[b],
            "hidRT8": np.ascontiguousarray(
                hs[b, q0:q0 + I_CORE].T.astype(f8)).reshape(H // P, P, I_CORE),
            "hid_rows": np.ascontiguousarray(
                hs[b, q0:q0 + I_CORE].reshape(I_CORE // P, P, H)),
            "bpe": np.ascontiguousarray(
                bpe[q0:q0 + I_CORE, :, b, :].transpose(0, 2, 1).astype(bpe_np_dt)),
            "WoT": WoT,
        }
        for w in ("Wq", "Wk", "Wv"):
            m[w + "8"] = W8[w]
        for bn in ("bq", "bk", "bv", "bo", "ln_gamma", "ln_beta"):
            m[bn] = np.ascontiguousarray(
                np.asarray(inputs[bn], dtype=np.float32).reshape(1, H))
        in_maps.append(m)
    return in_maps


def _install_ntff_shim():
    """The agent image's antenv lacks axon_hooks; recreate the NTFF profile
    hook via ctypes against libaxon_pjrt.so so trace=True yields
    exec_time_ns + a perfetto trace."""
    import sys as _sys
    if "antenv.axon_hooks" in _sys.modules:
        return
    import types, ctypes, contextlib
    so_path = "/opt/axon/libaxon_pjrt.so"
    mod = types.ModuleType("antenv.axon_hooks")
    _state = {}

    def get_axon_ntff_profile_hook():
        if "hook" in _state:
            return _state["hook"]
        try:
            lib = ctypes.CDLL(so_path)
            if not hasattr(lib, "axon_start_nrt_profile"):
                _state["hook"] = None
                return None
            lib.axon_start_nrt_profile.argtypes = [
                ctypes.POINTER(ctypes.c_int64), ctypes.c_size_t]
            lib.axon_start_nrt_profile.restype = ctypes.c_int64
            lib.axon_stop_nrt_profile.argtypes = [ctypes.c_char_p]
            lib.axon_stop_nrt_profile.restype = ctypes.c_int64
        except OSError:
            _state["hook"] = None
            return None

        @contextlib.contextmanager
        def _hook(output_dir, device_ids):
            import jax
            jax.devices()
            if device_ids:
                ids = (ctypes.c_int64 * len(device_ids))(*device_ids)
                rc = lib.axon_start_nrt_profile(ids, len(device_ids))
            else:
                rc = lib.axon_start_nrt_profile(None, 0)
            if rc != 0:
                raise RuntimeError(f"axon_start_nrt_profile rc={rc}")
            try:
                yield
            finally:
                n = lib.axon_stop_nrt_profile(str(output_dir).encode())
                print(f"ntff profile: {n} file(s) written to {output_dir}")

        _state["hook"] = _hook
        return _hook

    mod.get_axon_ntff_profile_hook = get_axon_ntff_profile_hook
    _sys.modules["antenv.axon_hooks"] = mod


def kernel(**inputs):
    from concourse.bass_utils import run_bass_kernel_spmd

    if os.environ.get("BASS_KERNEL_TRACE"):
        _install_ntff_shim()
        import concourse.bass_utils as _bu
        _bu.upload_artifacts = lambda tmpdir: f"file://{tmpdir}"

    if "nc" not in _COMPILED:
        _COMPILED["nc"] = build_kernel()
    nc = _COMPILED["nc"]
    in_maps = _shard_inputs(inputs)
    res = run_bass_kernel_spmd(nc, in_maps, core_ids=list(range(N_CORES)),
                               trace=bool(os.environ.get("BASS_KERNEL_TRACE")))
    _COMPILED["last_result"] = res
    out = np.zeros((B, S, H), dtype=np.float32)
    for c in range(N_CORES):
        b = c // 4
        q0 = (c % 4) * I_CORE
        out[b, q0:q0 + I_CORE] = np.asarray(
            res.results[c]["out"]).reshape(I_CORE, H)
    return out


# revision 48
# speedup vs baseline: 1.5268x; 1.0127x over previous
"""Distributed Trainium2 Bass kernel for BrosAttention (restructured v2).

B=2, S=1024, H=768, NH=12, DH=64:
  q,k,v = heads(hidden @ W.T + b)
  scores = q@k^T + einsum('bnid,bijd->bnij', q, bpe)   (bpe = bbox transposed)
  probs  = softmax(scores / 8)
  out    = LN(probs@v @ Wo.T + bo + hidden)

Sharding: 8 cores = 2 batches x 4 query-row blocks of 256 rows. Each core
reads only its slice of bbox_pos_emb, computes K/V for the full sequence of
its batch, writes a disjoint [256, 768] output slice. No collectives.

v2 structure (vs v1): transposed scores scoresT[j, (i)] per head; the bias
q.bpe is computed with qPair packed block-diagonally (col order 2n+s) so the
PE-transposed bias tiles are consumed RAW by the score add via strided APs
(no regroup copies). Softmax denominators come out of P@V via a 65th ones-
column on V (no ones-matmul reductions); 1/denom = exp(-ln(denom)) on ACT.
Full i=256 free dims everywhere.
"""

import os
import sys
import numpy as np

sys.path.insert(0, "/opt/trn_rl_repo")

B, S, H, NH, DH = 2, 1024, 768, 12, 64
EPS = 1e-12
P = 128
I_CORE = S * B // 8  # 256
N_CORES = 8

_COMPILED = {}

BPE_DT = "fp8"


def build_kernel(s=S, i_core=I_CORE, h=H, nh=NH, dh=DH):
    from contextlib import ExitStack
    from concourse import bacc, bass, mybir, tile

    f32 = mybir.dt.float32
    bf16 = mybir.dt.bfloat16
    fp8 = mybir.dt.float8e4
    bpe_dt = fp8 if BPE_DT == "fp8" else bf16
    Alu = mybir.AluOpType
    Act = mybir.ActivationFunctionType
    AxisX = mybir.AxisListType.X

    HC = h // P            # 6 hidden chunks
    SC = s // P            # 8 seq chunks (j)
    NPAIR = i_core // 2    # 128 i-pairs
    NOCT = i_core // 8     # 32 octos
    NOG = NOCT // 2        # 16 og-groups (2 octos = 16 i's each)
    NG = nh // 2           # 6 head pairs
    VH = h // 2            # 384

    nc = bacc.Bacc(None, target_bir_lowering=False, debug=False)

    # Steer the ACT table-load pass to the one set holding BOTH exp and ln
    # ("natural_log_exp_and_others"), so Exp/Ln alternation doesn't thrash
    # table loads. Indices stay aligned with act_info.json.
    from concourse import hw_specs
    try:
        tabs = hw_specs.get_activation_tables(nc.m.arch)
        if "natural_log_exp_and_others" in tabs:
            for name, funcs in tabs.items():
                if name != "natural_log_exp_and_others":
                    funcs.discard(mybir.ActivationFunctionType.Exp)
                    funcs.discard(mybir.ActivationFunctionType.Ln)
    except Exception:
        pass

    d_hidR = nc.declare_dram_parameter("hid_rows", [i_core // P, P, h], f32,
                                       isOutput=False)
    d_bpe = nc.declare_dram_parameter("bpe", [i_core, dh, s], bpe_dt, isOutput=False)
    d_W = {"Wo": nc.declare_dram_parameter("WoT", [HC, P, h], bf16,
                                           isOutput=False)}
    for w in ("Wq", "Wk", "Wv"):
        d_W[w] = nc.declare_dram_parameter(w + "8", [HC // 2, P, 2, h], fp8,
                                           isOutput=False)
    d_hidT8 = nc.declare_dram_parameter("hidT8", [HC, P, s], fp8, isOutput=False)
    d_hidRT8 = nc.declare_dram_parameter("hidRT8", [HC, P, i_core], fp8,
                                         isOutput=False)
    d_b = {bn: nc.declare_dram_parameter(bn, [1, h], f32, isOutput=False)
           for bn in ("bq", "bk", "bv", "bo", "ln_gamma", "ln_beta")}
    d_bT = {bn: nc.declare_dram_parameter(bn + "T", [P, HC], f32, isOutput=False)
            for bn in ("bq", "bk")}
    d_ident = nc.declare_dram_parameter("ident", [P, P], bf16, isOutput=False)
    d_out = nc.declare_dram_parameter("out", [i_core // P, P, h], f32, isOutput=True)

    with tile.TileContext(nc) as tc, ExitStack() as ctx:
        # ---------------- pools ----------------
        const_p = ctx.enter_context(tc.tile_pool(name="const", bufs=1))
        stat_p = ctx.enter_context(tc.tile_pool(name="stat", bufs=1))
        # psum: psQ "qk" [P,512]f32 (one bank) x6 bufs + pctx [65,4,256] x1 = 16KB
        psQ = ctx.enter_context(
            tc.tile_pool(name="psQ", bufs=6, space=bass.MemorySpace.PSUM))
        psC = ctx.enter_context(
            tc.tile_pool(name="psC", bufs=1, space=bass.MemorySpace.PSUM))

        def big():
            return psQ.tile([P, 512], f32, name="qk")
        bpe_p = ctx.enter_context(tc.tile_pool(name="bpe", bufs=3))
        b4_p = ctx.enter_context(tc.tile_pool(name="b4", bufs=2))
        sE_p = ctx.enter_context(tc.tile_pool(name="sE", bufs=2))
        pr_p = ctx.enter_context(tc.tile_pool(name="pr", bufs=2))
        y_p = ctx.enter_context(tc.tile_pool(name="y", bufs=1))

        # ---------------- constants ----------------
        ident_bf = const_p.tile([P, P], bf16)
        nc.sync.dma_start(ident_bf[:], d_ident[:])
        onesP = const_p.tile([P, P], bf16)
        nc.vector.memset(onesP[:], 1.0)
        ones_row = const_p.tile([1, s], bf16)
        nc.vector.memset(ones_row[:], 1.0)
        eps_t = const_p.tile([P, 1], f32)
        nc.vector.memset(eps_t[:], EPS)
        b_sb = {}
        b_bf = {}
        for bn in ("bq", "bk", "bv", "bo", "ln_gamma", "ln_beta"):
            b_sb[bn] = const_p.tile([1, h], f32, name=f"bias_{bn}")
            nc.sync.dma_start(b_sb[bn][:], d_b[bn][:])
            b_bf[bn] = const_p.tile([1, h], bf16, name=f"biasbf_{bn}")
            nc.vector.tensor_copy(b_bf[bn][:], b_sb[bn][:])

        bT = {}
        for bn in ("bq", "bk"):
            bT[bn] = const_p.tile([P, HC], f32, name=f"bT_{bn}")
            nc.sync.dma_start(bT[bn][:], d_bT[bn][:])
        bqTs = const_p.tile([P, HC], f32, name="bqTs")
        nc.vector.tensor_scalar(bqTs[:], bT["bq"][:], 0.125, None, Alu.mult)

        bcast = {}
        for bn in ("ln_gamma", "ln_beta"):
            t = stat_p.tile([P, h], bf16, name=f"bcast_{bn}")
            for c in range(HC):
                pbx = big()
                nc.tensor.matmul(pbx[:, 0:P], onesP[0:1, :],
                                 b_bf[bn][:, c * P:(c + 1) * P])
                nc.scalar.copy(t[:, c * P:(c + 1) * P], pbx[:, 0:P])
            bcast[bn] = t

        # ---------------- persistent activations ----------------
        hidR = stat_p.tile([P, i_core // P, h], f32)
        nc.sync.dma_start(hidR[:], d_hidR[:].transpose([1, 0, 2]))
        WoT = stat_p.tile([P, HC, h], bf16)
        nc.sync.dma_start(WoT[:], d_W["Wo"][:].transpose([1, 0, 2]))
        qT128 = stat_p.tile([P, nh, i_core], bf16)   # q/8 duplicated both halves
        qPair = stat_p.tile([P, NPAIR, 32], bpe_dt)  # block-diag, col = 2n+s
        kT128 = stat_p.tile([P, NG, s], bf16)
        v_sb = stat_p.tile([P, SC, nh, dh + 1], bf16)  # col dh = ones
        biasT = stat_p.tile([P, NOCT, SC, 4, 24], bf16)  # raw transposed bias
        ctxT = stat_p.tile([P, NG, i_core], bf16)

        # ------- phase E (projections) interleaved with bias generation -------
        with tc.tile_pool(name="early", bufs=1) as early_p, \
             tc.tile_pool(name="earlyW", bufs=1) as earlyW_p:
            hidRT = early_p.tile([P, HC, i_core], fp8)
            nc.sync.dma_start(hidRT[:], d_hidRT8[:].transpose([1, 0, 2]))
            hidT = early_p.tile([P, HC, s], fp8)
            nc.sync.dma_start(hidT[:], d_hidT8[:].transpose([1, 0, 2]))

            def load_WT(w):
                t = earlyW_p.tile([P, HC // 2, 2, h], fp8, name="WT")
                nc.sync.dma_start(t[:], d_W[w][:].transpose([1, 0, 2, 3]))
                return t

            # Q projection (transposed): qT = (Wq @ hidR^T + bq)/8, dup halves.
            # The 1/8 softmax scale is folded into q (QK and bias inherit it).
            WqT = load_WT("Wq")
            DR = mybir.MatmulPerfMode.DoubleRow
            for r in range(HC):
                pqt = big()
                pq = pqt[:, 0:i_core]
                for kcp in range(HC // 2):
                    nc.tensor.matmul(
                        pq, WqT[:, kcp, :, r * P:(r + 1) * P],
                        hidRT[:, 2 * kcp:2 * kcp + 2, :],
                        start=(kcp == 0), stop=(kcp == HC // 2 - 1), perf_mode=DR)
                for sub in range(2):
                    src = pqt[sub * dh:(sub + 1) * dh, 0:i_core]
                    bcol = bqTs[sub * dh:(sub + 1) * dh, r:r + 1]
                    nc.vector.tensor_scalar(qT128[0:dh, 2 * r + sub, :], src,
                                            0.125, bcol, Alu.mult, Alu.add)
                    nc.vector.tensor_scalar(qT128[dh:P, 2 * r + sub, :], src,
                                            0.125, bcol, Alu.mult, Alu.add)

            # qPair block-diag: rows 0:64 <- q even-i at cols 2n, rows 64:128 <-
            # q odd-i at cols 2n+1.
            nc.vector.memset(qPair[:], 0.0)
            nc.vector.tensor_copy(
                qPair[0:dh, :, 0:2 * nh:2],
                qT128[0:dh, :, 0::2].transpose([0, 2, 1]))
            nc.vector.tensor_copy(
                qPair[dh:P, :, 1:2 * nh:2],
                qT128[dh:P, :, 1::2].transpose([0, 2, 1]))
            nc.vector.memset(v_sb[:, :, :, dh:dh + 1], 1.0)

            WkT = load_WT("Wk")
            WvT = load_WT("Wv")

            def k_unit(r, jh):
                pkt = big()
                pk = pkt[:]
                for kcp in range(HC // 2):
                    nc.tensor.matmul(
                        pk, WkT[:, kcp, :, r * P:(r + 1) * P],
                        hidT[:, 2 * kcp:2 * kcp + 2,
                             jh * (s // 2):(jh + 1) * (s // 2)],
                        start=(kcp == 0), stop=(kcp == HC // 2 - 1), perf_mode=DR)
                nc.scalar.activation(
                    kT128[:, r, jh * (s // 2):(jh + 1) * (s // 2)], pk,
                    Act.Identity, bias=bT["bk"][:, r:r + 1])

            def v_unit(jc, vh):
                pvt = big()
                pv = pvt[:, 0:VH]
                for kcp in range(HC // 2):
                    nc.tensor.matmul(
                        pv,
                        hidT[:, 2 * kcp:2 * kcp + 2, jc * P:(jc + 1) * P],
                        WvT[:, kcp, :, vh * VH:(vh + 1) * VH],
                        start=(kcp == 0), stop=False, perf_mode=DR)
                nc.tensor.matmul(pv, ones_row[:, 0:P],
                                 b_bf["bv"][:, vh * VH:(vh + 1) * VH],
                                 start=False, stop=True)
                nc.scalar.copy(v_sb[:, jc, 6 * vh:6 * vh + 6, 0:dh], pv)

            def octo_unit(octo):
                # bias[n,i,j] = q[n,i,:].bpe[i,j,:] into rows 32*c4 + (2n+s),
                # j streaming; PE-transposed per j-chunk; stored RAW (dense).
                i0 = octo * 8
                bpeT = bpe_p.tile([P, 4, s], bpe_dt)
                nc.sync.dma_start(
                    bpeT[:],
                    d_bpe[i0:i0 + 8].rearrange("(a b) d j -> (b d) a j", a=4))
                pb_h = [big() for _ in range(2)]
                for c4 in range(4):
                    lhs = qPair[:, octo * 4 + c4, :]
                    for jh in range(2):
                        nc.tensor.matmul(
                            pb_h[jh][32 * c4:32 * c4 + 32, :], lhs,
                            bpeT[:, c4, jh * (s // 2):(jh + 1) * (s // 2)],
                            tile_position=(0, 32 * c4))
                b4 = b4_p.tile([P, s], bf16)
                nc.scalar.copy(b4[:, 0:s // 2], pb_h[0][:])
                nc.vector.tensor_copy(b4[:, s // 2:s], pb_h[1][:])
                ptb = big().bitcast(bf16).rearrange("p (j u) -> p j u", j=SC)
                for jc in range(SC):
                    nc.tensor.transpose(ptb[:, jc, :], b4[:, jc * P:(jc + 1) * P],
                                        ident_bf[:])
                psrc = ptb.rearrange("p j (c u) -> p j c u", c=4)[:, :, :, 0:24]
                if octo % 2 == 0:
                    nc.scalar.copy(biasT[:, octo, :, :, :], psrc)
                else:
                    nc.vector.tensor_copy(biasT[:, octo, :, :, :], psrc)

            proj_units = ([lambda r=r, jh=jh: k_unit(r, jh)
                           for r in range(HC) for jh in range(2)] +
                          [lambda jc=jc, vh=vh: v_unit(jc, vh)
                           for jc in range(SC) for vh in range(2)])
            pi = 0
            for octo in range(NOCT):
                octo_unit(octo)
                while pi * NOCT < (octo + 1) * len(proj_units):
                    proj_units[pi]()
                    pi += 1

        # ------- attention: 4-head groups, one-bank score tiles -------
        for g4 in range(nh // 4):
            pctx = psC.tile([dh + 1, 4, i_core], f32, name="pctx")
            for jc in range(SC):
                pqk_h = [big() for _ in range(4)]
                for hn in range(4):
                    n = 4 * g4 + hn
                    bb = dh * (n & 1)
                    nc.tensor.matmul(pqk_h[hn][:, 0:i_core],
                                     kT128[bb:bb + dh, n // 2, jc * P:(jc + 1) * P],
                                     qT128[bb:bb + dh, n, :])
                sE = sE_p.tile([P, 4, i_core], bf16)
                for hn in range(4):
                    n = 4 * g4 + hn
                    nc.vector.tensor_tensor(
                        sE[:, hn, :].rearrange("p (o c u) -> p o c u", o=NOCT, c=4),
                        pqk_h[hn][:, 0:i_core].rearrange(
                            "p (o c u) -> p o c u", o=NOCT, c=4),
                        biasT[:, :, jc, :, 2 * n:2 * n + 2], Alu.add)
                probsT = pr_p.tile([P, 4, i_core], bf16)
                nc.scalar.activation(probsT[:], sE[:], Act.Exp)
                for hn in range(4):
                    n = 4 * g4 + hn
                    nc.tensor.matmul(pctx[:, hn, :], v_sb[:, jc, n, :],
                                     probsT[:, hn, :],
                                     start=(jc == 0), stop=(jc == SC - 1),
                                     skip_group_check=True)
            # evacuate ctx + denominators; 1/denom = exp(-ln(denom)) broadcast
            # to 128 partitions via K=1 matmuls, pipelined with the next group.
            denomS = y_p.tile([1, 4, i_core], bf16, name="dnm")
            for hn in range(4):
                n = 4 * g4 + hn
                r0 = dh * (n & 1)
                nc.vector.tensor_copy(ctxT[r0:r0 + dh, n // 2, :],
                                      pctx[0:dh, hn, :])
                nc.vector.tensor_copy(denomS[:, hn, :], pctx[dh:dh + 1, hn, :])
            prec = big()
            nc.tensor.matmul(prec[:], onesP[0:1, :],
                             denomS[:, 0:2, :].rearrange("p a b -> p (a b)"))
            prec2 = big()
            nc.tensor.matmul(prec2[:], onesP[0:1, :],
                             denomS[:, 2:4, :].rearrange("p a b -> p (a b)"))
            lgr = y_p.tile([P, i_core * 2], bf16, name="lgrA")
            nc.scalar.activation(lgr[:], prec[:], Act.Ln)
            lgr2 = y_p.tile([P, i_core * 2], bf16, name="lgrB")
            nc.scalar.activation(lgr2[:], prec2[:], Act.Ln)
            for gh, lg in ((0, lgr), (1, lgr2)):
                g = 2 * g4 + gh
                recB = y_p.tile([P, 2, i_core], bf16, name=f"recB{g % 2}")
                nc.scalar.activation(recB[:].rearrange("p a b -> p (a b)"), lg[:],
                                     Act.Exp, scale=-1.0)
                nc.vector.tensor_tensor(ctxT[0:dh, g, :], ctxT[0:dh, g, :],
                                        recB[0:dh, 0, :], Alu.mult)
                nc.vector.tensor_tensor(ctxT[dh:P, g, :], ctxT[dh:P, g, :],
                                        recB[dh:P, 1, :], Alu.mult)

        # ---------------- O-proj + residual + LN ----------------
        for half in range(2):
            i0 = half * P
            pys = [big() for _ in range(2)]
            for vh in range(2):
                for kc in range(HC):
                    nc.tensor.matmul(pys[vh][:, 0:VH], ctxT[:, kc, i0:i0 + P],
                                     WoT[:, kc, vh * VH:(vh + 1) * VH],
                                     start=(kc == 0), stop=False)
                nc.tensor.matmul(pys[vh][:, 0:VH], ones_row[:, 0:P],
                                 b_bf["bo"][:, vh * VH:(vh + 1) * VH],
                                 start=False, stop=True)
            y = y_p.tile([P, h], f32)
            for vh in range(2):
                nc.vector.tensor_tensor(y[:, vh * VH:(vh + 1) * VH],
                                        pys[vh][:, 0:VH],
                                        hidR[:, half, vh * VH:(vh + 1) * VH],
                                        Alu.add)
            mu = y_p.tile([P, 1], f32)
            nc.vector.tensor_reduce(mu[:], y[:], AxisX, Alu.add)
            nc.vector.tensor_scalar(mu[:], mu[:], 1.0 / h, None, Alu.mult)
            yc = y_p.tile([P, h], f32)
            nc.vector.tensor_scalar(yc[:], y[:], mu[:], None, Alu.subtract)
            ssq = y_p.tile([P, 1], f32)
            nc.scalar.activation(y[:], yc[:], Act.Square, accum_out=ssq[:])
            std = y_p.tile([P, 1], f32)
            nc.scalar.activation(std[:], ssq[:], Act.Sqrt,
                                 scale=1.0 / h, bias=eps_t[:])
            rstd = y_p.tile([P, 1], f32)
            nc.vector.reciprocal(rstd[:], std[:])
            o1 = y_p.tile([P, h], f32)
            nc.vector.scalar_tensor_tensor(o1[:], yc[:], rstd[:],
                                           bcast["ln_gamma"][:],
                                           Alu.mult, Alu.mult)
            nc.vector.tensor_tensor(o1[:], o1[:], bcast["ln_beta"][:], Alu.add)
            nc.sync.dma_start(d_out[half], o1[:])

    nc.compile()
    return nc


def _shard_inputs(inputs):
    import ml_dtypes
    bf = ml_dtypes.bfloat16
    f8 = ml_dtypes.float8_e4m3
    bpe_np_dt = f8 if BPE_DT == "fp8" else bf
    hs = np.ascontiguousarray(np.asarray(inputs["hidden_states"]), dtype=np.float32)
    bpe = np.asarray(inputs["bbox_pos_emb"])
    ident = np.eye(P, dtype=np.float32).astype(bf)
    hsT8 = {b: np.ascontiguousarray(hs[b].T.astype(f8)).reshape(H // P, P, S)
            for b in range(B)}
    WoT = np.ascontiguousarray(
        np.asarray(inputs["Wo"], dtype=np.float32).T.astype(bf)).reshape(
            H // P, P, H)
    W8 = {w: np.ascontiguousarray(
             np.asarray(inputs[w], dtype=np.float32).T.astype(f8).reshape(
                 H // 256, 2, P, H).transpose(0, 2, 1, 3))
          for w in ("Wq", "Wk", "Wv")}
    in_maps = []
    for c in range(N_CORES):
        b = c // 4
        q0 = (c % 4) * I_CORE
        m = {
            "ident": ident,
            "hidT8": hsT8[b],
            "hidRT8": np.ascontiguousarray(
                hs[b, q0:q0 + I_CORE].T.astype(f8)).reshape(H // P, P, I_CORE),
            "hid_rows": np.ascontiguousarray(
                hs[b, q0:q0 + I_CORE].reshape(I_CORE // P, P, H)),
            "bpe": np.ascontiguousarray(
                bpe[q0:q0 + I_CORE, :, b, :].transpose(0, 2, 1).astype(bpe_np_dt)),
            "WoT": WoT,
        }
        for w in ("Wq", "Wk", "Wv"):
            m[w + "8"] = W8[w]
        for bn in ("bq", "bk", "bv", "bo", "ln_gamma", "ln_beta"):
            m[bn] = np.ascontiguousarray(
                np.asarray(inputs[bn], dtype=np.float32).reshape(1, H))
        for bn in ("bq", "bk"):
            m[bn + "T"] = np.ascontiguousarray(
                np.asarray(inputs[bn], dtype=np.float32).reshape(H // P, P).T)
        in_maps.append(m)
    return in_maps


def _install_ntff_shim():
    """The agent image's antenv lacks axon_hooks; recreate the NTFF profile
    hook via ctypes against libaxon_pjrt.so so trace=True yields
    exec_time_ns + a perfetto trace."""
    import sys as _sys
    if "antenv.axon_hooks" in _sys.modules:
        return
    import types, ctypes, contextlib
    so_path = "/opt/axon/libaxon_pjrt.so"
    mod = types.ModuleType("antenv.axon_hooks")
    _state = {}

    def get_axon_ntff_profile_hook():
        if "hook" in _state:
            return _state["hook"]
        try:
            lib = ctypes.CDLL(so_path)
            if not hasattr(lib, "axon_start_nrt_profile"):
                _state["hook"] = None
                return None
            lib.axon_start_nrt_profile.argtypes = [
                ctypes.POINTER(ctypes.c_int64), ctypes.c_size_t]
            lib.axon_start_nrt_profile.restype = ctypes.c_int64
            lib.axon_stop_nrt_profile.argtypes = [ctypes.c_char_p]
            lib.axon_stop_nrt_profile.restype = ctypes.c_int64
        except OSError:
            _state["hook"] = None
            return None

        @contextlib.contextmanager
        def _hook(output_dir, device_ids):
            import jax
            jax.devices()
            if device_ids:
                ids = (ctypes.c_int64 * len(device_ids))(*device_ids)
                rc = lib.axon_start_nrt_profile(ids, len(device_ids))
            else:
                rc = lib.axon_start_nrt_profile(None, 0)
            if rc != 0:
                raise RuntimeError(f"axon_start_nrt_profile rc={rc}")
            try:
                yield
            finally:
                n = lib.axon_stop_nrt_profile(str(output_dir).encode())
                print(f"ntff profile: {n} file(s) written to {output_dir}")

        _state["hook"] = _hook
        return _hook

    mod.get_axon_ntff_profile_hook = get_axon_ntff_profile_hook
    _sys.modules["antenv.axon_hooks"] = mod


def kernel(**inputs):
    from concourse.bass_utils import run_bass_kernel_spmd

    if os.environ.get("BASS_KERNEL_TRACE"):
        _install_ntff_shim()
        import concourse.bass_utils as _bu
        _bu.upload_artifacts = lambda tmpdir: f"file://{tmpdir}"

    if "nc" not in _COMPILED:
        _COMPILED["nc"] = build_kernel()
    nc = _COMPILED["nc"]
    in_maps = _shard_inputs(inputs)
    res = run_bass_kernel_spmd(nc, in_maps, core_ids=list(range(N_CORES)),
                               trace=bool(os.environ.get("BASS_KERNEL_TRACE")))
    _COMPILED["last_result"] = res
    out = np.zeros((B, S, H), dtype=np.float32)
    for c in range(N_CORES):
        b = c // 4
        q0 = (c % 4) * I_CORE
        out[b, q0:q0 + I_CORE] = np.asarray(
            res.results[c]["out"]).reshape(I_CORE, H)
    return out


# revision 49
# speedup vs baseline: 1.6277x; 1.0661x over previous
"""Distributed Trainium2 Bass kernel for BrosAttention (restructured v2).

B=2, S=1024, H=768, NH=12, DH=64:
  q,k,v = heads(hidden @ W.T + b)
  scores = q@k^T + einsum('bnid,bijd->bnij', q, bpe)   (bpe = bbox transposed)
  probs  = softmax(scores / 8)
  out    = LN(probs@v @ Wo.T + bo + hidden)

Sharding: 8 cores = 2 batches x 4 query-row blocks of 256 rows. Each core
reads only its slice of bbox_pos_emb, computes K/V for the full sequence of
its batch, writes a disjoint [256, 768] output slice. No collectives.

Structure: transposed scores scoresT[j, i] per head. The bias q.bpe is
computed from fp8 bpe (host-cast) with qPair packed block-diagonally
(col order 2n+s, 4 concurrent PE column-tiles), PE-transposed per j-chunk,
and stored RAW so the score add consumes it via strided APs (no regroup
copies). Q/K/V projections run in fp8 DoubleRow (256-contraction);
projections and bias generation are emitted interleaved so the bpe stream
overlaps compute. Softmax denominators come from P@V via a 65th ones-column
on V; 1/denom = exp(-ln(denom)) on ACT (table-load steering keeps Exp+Ln in
one ACT table set). The 1/8 softmax scale is folded into q. Full i=256 free
dims; one-bank (2KB) psum tiles throughout to avoid cross-engine PSUM bank
collisions.
"""

import os
import sys
import numpy as np

sys.path.insert(0, "/opt/trn_rl_repo")

B, S, H, NH, DH = 2, 1024, 768, 12, 64
EPS = 1e-12
P = 128
I_CORE = S * B // 8  # 256
N_CORES = 8

_COMPILED = {}

BPE_DT = "fp8"


def build_kernel(s=S, i_core=I_CORE, h=H, nh=NH, dh=DH):
    from contextlib import ExitStack
    from concourse import bacc, bass, mybir, tile

    f32 = mybir.dt.float32
    bf16 = mybir.dt.bfloat16
    fp8 = mybir.dt.float8e4
    bpe_dt = fp8 if BPE_DT == "fp8" else bf16
    Alu = mybir.AluOpType
    Act = mybir.ActivationFunctionType
    AxisX = mybir.AxisListType.X

    HC = h // P            # 6 hidden chunks
    SC = s // P            # 8 seq chunks (j)
    NPAIR = i_core // 2    # 128 i-pairs
    NOCT = i_core // 8     # 32 octos
    NOG = NOCT // 2        # 16 og-groups (2 octos = 16 i's each)
    NG = nh // 2           # 6 head pairs
    VH = h // 2            # 384

    nc = bacc.Bacc(None, target_bir_lowering=False, debug=False)

    # Steer the ACT table-load pass to the one set holding BOTH exp and ln
    # ("natural_log_exp_and_others"), so Exp/Ln alternation doesn't thrash
    # table loads. Indices stay aligned with act_info.json.
    from concourse import hw_specs
    try:
        tabs = hw_specs.get_activation_tables(nc.m.arch)
        if "natural_log_exp_and_others" in tabs:
            for name, funcs in tabs.items():
                if name != "natural_log_exp_and_others":
                    funcs.discard(mybir.ActivationFunctionType.Exp)
                    funcs.discard(mybir.ActivationFunctionType.Ln)
    except Exception:
        pass

    d_hidR = nc.declare_dram_parameter("hid_rows", [i_core // P, P, h], f32,
                                       isOutput=False)
    d_bpe = nc.declare_dram_parameter("bpe", [i_core, dh, s], bpe_dt, isOutput=False)
    d_W = {"Wo": nc.declare_dram_parameter("WoT", [HC, P, h], bf16,
                                           isOutput=False)}
    for w in ("Wq", "Wk", "Wv"):
        d_W[w] = nc.declare_dram_parameter(w + "8", [HC // 2, P, 2, h], fp8,
                                           isOutput=False)
    d_hidT8 = nc.declare_dram_parameter("hidT8", [HC, P, s], fp8, isOutput=False)
    d_hidRT8 = nc.declare_dram_parameter("hidRT8", [HC, P, i_core], fp8,
                                         isOutput=False)
    d_b = {bn: nc.declare_dram_parameter(bn, [1, h], f32, isOutput=False)
           for bn in ("bq", "bk", "bv", "bo", "ln_gamma", "ln_beta")}
    d_bT = {bn: nc.declare_dram_parameter(bn + "T", [P, HC], f32, isOutput=False)
            for bn in ("bq", "bk")}
    d_ident = nc.declare_dram_parameter("ident", [P, P], bf16, isOutput=False)
    d_out = nc.declare_dram_parameter("out", [i_core // P, P, h], f32, isOutput=True)

    with tile.TileContext(nc) as tc, ExitStack() as ctx:
        # ---------------- pools ----------------
        const_p = ctx.enter_context(tc.tile_pool(name="const", bufs=1))
        stat_p = ctx.enter_context(tc.tile_pool(name="stat", bufs=1))
        # psum: psQ "qk" [P,512]f32 (one bank) x6 bufs + pctx [65,4,256] x1 = 16KB
        psQ = ctx.enter_context(
            tc.tile_pool(name="psQ", bufs=6, space=bass.MemorySpace.PSUM))
        psC = ctx.enter_context(
            tc.tile_pool(name="psC", bufs=1, space=bass.MemorySpace.PSUM))

        def big():
            return psQ.tile([P, 512], f32, name="qk")
        bpe_p = ctx.enter_context(tc.tile_pool(name="bpe", bufs=3))
        b4_p = ctx.enter_context(tc.tile_pool(name="b4", bufs=2))
        sE_p = ctx.enter_context(tc.tile_pool(name="sE", bufs=2))
        pr_p = ctx.enter_context(tc.tile_pool(name="pr", bufs=2))
        y_p = ctx.enter_context(tc.tile_pool(name="y", bufs=1))

        # ---------------- constants ----------------
        ident_bf = const_p.tile([P, P], bf16)
        nc.sync.dma_start(ident_bf[:], d_ident[:])
        onesP = const_p.tile([P, P], bf16)
        nc.vector.memset(onesP[:], 1.0)
        ones_row = const_p.tile([1, s], bf16)
        nc.vector.memset(ones_row[:], 1.0)
        eps_t = const_p.tile([P, 1], f32)
        nc.vector.memset(eps_t[:], EPS)
        b_sb = {}
        b_bf = {}
        for bn in ("bq", "bk", "bv", "bo", "ln_gamma", "ln_beta"):
            b_sb[bn] = const_p.tile([1, h], f32, name=f"bias_{bn}")
            nc.sync.dma_start(b_sb[bn][:], d_b[bn][:])
            b_bf[bn] = const_p.tile([1, h], bf16, name=f"biasbf_{bn}")
            nc.vector.tensor_copy(b_bf[bn][:], b_sb[bn][:])

        bT = {}
        for bn in ("bq", "bk"):
            bT[bn] = const_p.tile([P, HC], f32, name=f"bT_{bn}")
            nc.sync.dma_start(bT[bn][:], d_bT[bn][:])
        bqTs = const_p.tile([P, HC], f32, name="bqTs")
        nc.vector.tensor_scalar(bqTs[:], bT["bq"][:], 0.125, None, Alu.mult)

        bcast = {}
        for bn in ("ln_gamma", "ln_beta"):
            t = stat_p.tile([P, h], bf16, name=f"bcast_{bn}")
            for c in range(HC):
                pbx = big()
                nc.tensor.matmul(pbx[:, 0:P], onesP[0:1, :],
                                 b_bf[bn][:, c * P:(c + 1) * P])
                nc.scalar.copy(t[:, c * P:(c + 1) * P], pbx[:, 0:P])
            bcast[bn] = t

        # ---------------- persistent activations ----------------
        hidR = stat_p.tile([P, i_core // P, h], f32)
        nc.sync.dma_start(hidR[:], d_hidR[:].transpose([1, 0, 2]))
        WoT = stat_p.tile([P, HC, h], bf16)
        nc.sync.dma_start(WoT[:], d_W["Wo"][:].transpose([1, 0, 2]))
        qT128 = stat_p.tile([P, nh, i_core], bf16)   # q/8 duplicated both halves
        qPair = stat_p.tile([P, NPAIR, 32], bpe_dt)  # block-diag, col = 2n+s
        kT128 = stat_p.tile([P, NG, s], bf16)
        v_sb = stat_p.tile([P, SC, nh, dh + 1], bf16)  # col dh = ones
        biasT = stat_p.tile([P, NOCT, SC, 4, 24], bf16)  # raw transposed bias
        ctxT = stat_p.tile([P, NG, i_core], bf16)

        # ------- phase E (projections) interleaved with bias generation -------
        with tc.tile_pool(name="early", bufs=1) as early_p, \
             tc.tile_pool(name="earlyW", bufs=1) as earlyW_p:
            hidRT = early_p.tile([P, HC, i_core], fp8)
            nc.sync.dma_start(hidRT[:], d_hidRT8[:].transpose([1, 0, 2]))
            hidT = early_p.tile([P, HC, s], fp8)
            nc.sync.dma_start(hidT[:], d_hidT8[:].transpose([1, 0, 2]))

            def load_WT(w):
                t = earlyW_p.tile([P, HC // 2, 2, h], fp8, name="WT")
                nc.sync.dma_start(t[:], d_W[w][:].transpose([1, 0, 2, 3]))
                return t

            # Q projection (transposed): qT = (Wq @ hidR^T + bq)/8, dup halves.
            # The 1/8 softmax scale is folded into q (QK and bias inherit it).
            WqT = load_WT("Wq")
            DR = mybir.MatmulPerfMode.DoubleRow
            for r in range(HC):
                pqt = big()
                pq = pqt[:, 0:i_core]
                for kcp in range(HC // 2):
                    nc.tensor.matmul(
                        pq, WqT[:, kcp, :, r * P:(r + 1) * P],
                        hidRT[:, 2 * kcp:2 * kcp + 2, :],
                        start=(kcp == 0), stop=(kcp == HC // 2 - 1), perf_mode=DR)
                for sub in range(2):
                    src = pqt[sub * dh:(sub + 1) * dh, 0:i_core]
                    bcol = bqTs[sub * dh:(sub + 1) * dh, r:r + 1]
                    nc.vector.tensor_scalar(qT128[0:dh, 2 * r + sub, :], src,
                                            0.125, bcol, Alu.mult, Alu.add)
                    nc.vector.tensor_scalar(qT128[dh:P, 2 * r + sub, :], src,
                                            0.125, bcol, Alu.mult, Alu.add)

            # qPair block-diag: rows 0:64 <- q even-i at cols 2n, rows 64:128 <-
            # q odd-i at cols 2n+1.
            nc.vector.memset(qPair[:], 0.0)
            nc.vector.tensor_copy(
                qPair[0:dh, :, 0:2 * nh:2],
                qT128[0:dh, :, 0::2].transpose([0, 2, 1]))
            nc.vector.tensor_copy(
                qPair[dh:P, :, 1:2 * nh:2],
                qT128[dh:P, :, 1::2].transpose([0, 2, 1]))
            nc.vector.memset(v_sb[:, :, :, dh:dh + 1], 1.0)

            WkT = load_WT("Wk")
            WvT = load_WT("Wv")

            def k_unit(r, jh):
                pkt = big()
                pk = pkt[:]
                for kcp in range(HC // 2):
                    nc.tensor.matmul(
                        pk, WkT[:, kcp, :, r * P:(r + 1) * P],
                        hidT[:, 2 * kcp:2 * kcp + 2,
                             jh * (s // 2):(jh + 1) * (s // 2)],
                        start=(kcp == 0), stop=(kcp == HC // 2 - 1), perf_mode=DR)
                nc.scalar.activation(
                    kT128[:, r, jh * (s // 2):(jh + 1) * (s // 2)], pk,
                    Act.Identity, bias=bT["bk"][:, r:r + 1])

            def v_unit(jc, vh):
                pvt = big()
                pv = pvt[:, 0:VH]
                for kcp in range(HC // 2):
                    nc.tensor.matmul(
                        pv,
                        hidT[:, 2 * kcp:2 * kcp + 2, jc * P:(jc + 1) * P],
                        WvT[:, kcp, :, vh * VH:(vh + 1) * VH],
                        start=(kcp == 0), stop=False, perf_mode=DR)
                nc.tensor.matmul(pv, ones_row[:, 0:P],
                                 b_bf["bv"][:, vh * VH:(vh + 1) * VH],
                                 start=False, stop=True)
                nc.scalar.copy(v_sb[:, jc, 6 * vh:6 * vh + 6, 0:dh], pv)

            def octo_unit(octo):
                # bias[n,i,j] = q[n,i,:].bpe[i,j,:] into rows 32*c4 + (2n+s),
                # j streaming; PE-transposed per j-chunk; stored RAW (dense).
                i0 = octo * 8
                bpeT = bpe_p.tile([P, 4, s], bpe_dt)
                nc.sync.dma_start(
                    bpeT[:],
                    d_bpe[i0:i0 + 8].rearrange("(a b) d j -> (b d) a j", a=4))
                pb_h = [big() for _ in range(2)]
                for c4 in range(4):
                    lhs = qPair[:, octo * 4 + c4, :]
                    for jh in range(2):
                        nc.tensor.matmul(
                            pb_h[jh][32 * c4:32 * c4 + 32, :], lhs,
                            bpeT[:, c4, jh * (s // 2):(jh + 1) * (s // 2)],
                            tile_position=(0, 32 * c4))
                b4 = b4_p.tile([P, s], bf16)
                nc.scalar.copy(b4[:, 0:s // 2], pb_h[0][:])
                nc.vector.tensor_copy(b4[:, s // 2:s], pb_h[1][:])
                ptb = big().bitcast(bf16).rearrange("p (j u) -> p j u", j=SC)
                for jc in range(SC):
                    nc.tensor.transpose(ptb[:, jc, :], b4[:, jc * P:(jc + 1) * P],
                                        ident_bf[:])
                psrc = ptb.rearrange("p j (c u) -> p j c u", c=4)[:, :, :, 0:24]
                if octo % 2 == 0:
                    nc.scalar.copy(biasT[:, octo, :, :, :], psrc)
                else:
                    nc.vector.tensor_copy(biasT[:, octo, :, :, :], psrc)

            proj_units = ([lambda r=r, jh=jh: k_unit(r, jh)
                           for r in range(HC) for jh in range(2)] +
                          [lambda jc=jc, vh=vh: v_unit(jc, vh)
                           for jc in range(SC) for vh in range(2)])
            pi = 0
            for octo in range(NOCT):
                octo_unit(octo)
                while pi * NOCT < (octo + 1) * len(proj_units):
                    proj_units[pi]()
                    pi += 1

        # ------- attention: 4-head groups, one-bank score tiles -------
        for g4 in range(nh // 4):
            pctx = psC.tile([dh + 1, 4, i_core], f32, name="pctx")
            for jc in range(SC):
                pqk_h = [big() for _ in range(4)]
                for hn in range(4):
                    n = 4 * g4 + hn
                    bb = dh * (n & 1)
                    nc.tensor.matmul(pqk_h[hn][:, 0:i_core],
                                     kT128[bb:bb + dh, n // 2, jc * P:(jc + 1) * P],
                                     qT128[bb:bb + dh, n, :])
                sE = sE_p.tile([P, 4, i_core], bf16)
                for hn in range(4):
                    n = 4 * g4 + hn
                    nc.vector.tensor_tensor(
                        sE[:, hn, :].rearrange("p (o c u) -> p o c u", o=NOCT, c=4),
                        pqk_h[hn][:, 0:i_core].rearrange(
                            "p (o c u) -> p o c u", o=NOCT, c=4),
                        biasT[:, :, jc, :, 2 * n:2 * n + 2], Alu.add)
                probsT = pr_p.tile([P, 4, i_core], bf16)
                nc.scalar.activation(probsT[:], sE[:], Act.Exp)
                for hn in range(4):
                    n = 4 * g4 + hn
                    nc.tensor.matmul(pctx[:, hn, :], v_sb[:, jc, n, :],
                                     probsT[:, hn, :],
                                     start=(jc == 0), stop=(jc == SC - 1),
                                     skip_group_check=True)
            # evacuate ctx + denominators; 1/denom = exp(-ln(denom)) broadcast
            # to 128 partitions via K=1 matmuls, pipelined with the next group.
            denomS = y_p.tile([1, 4, i_core], bf16, name="dnm")
            for hn in range(4):
                n = 4 * g4 + hn
                r0 = dh * (n & 1)
                nc.vector.tensor_copy(ctxT[r0:r0 + dh, n // 2, :],
                                      pctx[0:dh, hn, :])
                nc.vector.tensor_copy(denomS[:, hn, :], pctx[dh:dh + 1, hn, :])
            prec = big()
            nc.tensor.matmul(prec[:], onesP[0:1, :],
                             denomS[:, 0:2, :].rearrange("p a b -> p (a b)"))
            prec2 = big()
            nc.tensor.matmul(prec2[:], onesP[0:1, :],
                             denomS[:, 2:4, :].rearrange("p a b -> p (a b)"))
            lgr = y_p.tile([P, i_core * 2], bf16, name="lgrA")
            nc.scalar.activation(lgr[:], prec[:], Act.Ln)
            lgr2 = y_p.tile([P, i_core * 2], bf16, name="lgrB")
            nc.scalar.activation(lgr2[:], prec2[:], Act.Ln)
            for gh, lg in ((0, lgr), (1, lgr2)):
                g = 2 * g4 + gh
                recB = y_p.tile([P, 2, i_core], bf16, name=f"recB{g % 2}")
                nc.scalar.activation(recB[:].rearrange("p a b -> p (a b)"), lg[:],
                                     Act.Exp, scale=-1.0)
                nc.vector.tensor_tensor(ctxT[0:dh, g, :], ctxT[0:dh, g, :],
                                        recB[0:dh, 0, :], Alu.mult)
                nc.vector.tensor_tensor(ctxT[dh:P, g, :], ctxT[dh:P, g, :],
                                        recB[dh:P, 1, :], Alu.mult)

        # ---------------- O-proj + residual + LN ----------------
        for half in range(2):
            i0 = half * P
            pys = [big() for _ in range(2)]
            for vh in range(2):
                for kc in range(HC):
                    nc.tensor.matmul(pys[vh][:, 0:VH], ctxT[:, kc, i0:i0 + P],
                                     WoT[:, kc, vh * VH:(vh + 1) * VH],
                                     start=(kc == 0), stop=False)
                nc.tensor.matmul(pys[vh][:, 0:VH], ones_row[:, 0:P],
                                 b_bf["bo"][:, vh * VH:(vh + 1) * VH],
                                 start=False, stop=True)
            y = y_p.tile([P, h], f32)
            for vh in range(2):
                nc.vector.tensor_tensor(y[:, vh * VH:(vh + 1) * VH],
                                        pys[vh][:, 0:VH],
                                        hidR[:, half, vh * VH:(vh + 1) * VH],
                                        Alu.add)
            mu = y_p.tile([P, 1], f32)
            nc.vector.tensor_reduce(mu[:], y[:], AxisX, Alu.add)
            nc.vector.tensor_scalar(mu[:], mu[:], 1.0 / h, None, Alu.mult)
            yc = y_p.tile([P, h], f32)
            nc.vector.tensor_scalar(yc[:], y[:], mu[:], None, Alu.subtract)
            ssq = y_p.tile([P, 1], f32)
            nc.scalar.activation(y[:], yc[:], Act.Square, accum_out=ssq[:])
            std = y_p.tile([P, 1], f32)
            nc.scalar.activation(std[:], ssq[:], Act.Sqrt,
                                 scale=1.0 / h, bias=eps_t[:])
            rstd = y_p.tile([P, 1], f32)
            nc.vector.reciprocal(rstd[:], std[:])
            o1 = y_p.tile([P, h], f32)
            nc.vector.scalar_tensor_tensor(o1[:], yc[:], rstd[:],
                                           bcast["ln_gamma"][:],
                                           Alu.mult, Alu.mult)
            nc.vector.tensor_tensor(o1[:], o1[:], bcast["ln_beta"][:], Alu.add)
            nc.sync.dma_start(d_out[half], o1[:])

    nc.compile()
    return nc


def _shard_inputs(inputs):
    import ml_dtypes
    bf = ml_dtypes.bfloat16
    f8 = ml_dtypes.float8_e4m3
    bpe_np_dt = f8 if BPE_DT == "fp8" else bf
    hs = np.ascontiguousarray(np.asarray(inputs["hidden_states"]), dtype=np.float32)
    bpe = np.asarray(inputs["bbox_pos_emb"])
    ident = np.eye(P, dtype=np.float32).astype(bf)
    hsT8 = {b: np.ascontiguousarray(hs[b].T.astype(f8)).reshape(H // P, P, S)
            for b in range(B)}
    WoT = np.ascontiguousarray(
        np.asarray(inputs["Wo"], dtype=np.float32).T.astype(bf)).reshape(
            H // P, P, H)
    W8 = {w: np.ascontiguousarray(
             np.asarray(inputs[w], dtype=np.float32).T.astype(f8).reshape(
                 H // 256, 2, P, H).transpose(0, 2, 1, 3))
          for w in ("Wq", "Wk", "Wv")}
    in_maps = []
    for c in range(N_CORES):
        b = c // 4
        q0 = (c % 4) * I_CORE
        m = {
            "ident": ident,
            "hidT8": hsT8[b],
            "hidRT8": np.ascontiguousarray(
                hs[b, q0:q0 + I_CORE].T.astype(f8)).reshape(H // P, P, I_CORE),
            "hid_rows": np.ascontiguousarray(
                hs[b, q0:q0 + I_CORE].reshape(I_CORE // P, P, H)),
            "bpe": np.ascontiguousarray(
                bpe[q0:q0 + I_CORE, :, b, :].transpose(0, 2, 1).astype(bpe_np_dt)),
            "WoT": WoT,
        }
        for w in ("Wq", "Wk", "Wv"):
            m[w + "8"] = W8[w]
        for bn in ("bq", "bk", "bv", "bo", "ln_gamma", "ln_beta"):
            m[bn] = np.ascontiguousarray(
                np.asarray(inputs[bn], dtype=np.float32).reshape(1, H))
        for bn in ("bq", "bk"):
            m[bn + "T"] = np.ascontiguousarray(
                np.asarray(inputs[bn], dtype=np.float32).reshape(H // P, P).T)
        in_maps.append(m)
    return in_maps


def _install_ntff_shim():
    """The agent image's antenv lacks axon_hooks; recreate the NTFF profile
    hook via ctypes against libaxon_pjrt.so so trace=True yields
    exec_time_ns + a perfetto trace."""
    import sys as _sys
    if "antenv.axon_hooks" in _sys.modules:
        return
    import types, ctypes, contextlib
    so_path = "/opt/axon/libaxon_pjrt.so"
    mod = types.ModuleType("antenv.axon_hooks")
    _state = {}

    def get_axon_ntff_profile_hook():
        if "hook" in _state:
            return _state["hook"]
        try:
            lib = ctypes.CDLL(so_path)
            if not hasattr(lib, "axon_start_nrt_profile"):
                _state["hook"] = None
                return None
            lib.axon_start_nrt_profile.argtypes = [
                ctypes.POINTER(ctypes.c_int64), ctypes.c_size_t]
            lib.axon_start_nrt_profile.restype = ctypes.c_int64
            lib.axon_stop_nrt_profile.argtypes = [ctypes.c_char_p]
            lib.axon_stop_nrt_profile.restype = ctypes.c_int64
        except OSError:
            _state["hook"] = None
            return None

        @contextlib.contextmanager
        def _hook(output_dir, device_ids):
            import jax
            jax.devices()
            if device_ids:
                ids = (ctypes.c_int64 * len(device_ids))(*device_ids)
                rc = lib.axon_start_nrt_profile(ids, len(device_ids))
            else:
                rc = lib.axon_start_nrt_profile(None, 0)
            if rc != 0:
                raise RuntimeError(f"axon_start_nrt_profile rc={rc}")
            try:
                yield
            finally:
                n = lib.axon_stop_nrt_profile(str(output_dir).encode())
                print(f"ntff profile: {n} file(s) written to {output_dir}")

        _state["hook"] = _hook
        return _hook

    mod.get_axon_ntff_profile_hook = get_axon_ntff_profile_hook
    _sys.modules["antenv.axon_hooks"] = mod


def kernel(**inputs):
    from concourse.bass_utils import run_bass_kernel_spmd

    if os.environ.get("BASS_KERNEL_TRACE"):
        _install_ntff_shim()
        import concourse.bass_utils as _bu
        _bu.upload_artifacts = lambda tmpdir: f"file://{tmpdir}"

    if "nc" not in _COMPILED:
        _COMPILED["nc"] = build_kernel()
    nc = _COMPILED["nc"]
    in_maps = _shard_inputs(inputs)
    res = run_bass_kernel_spmd(nc, in_maps, core_ids=list(range(N_CORES)),
                               trace=bool(os.environ.get("BASS_KERNEL_TRACE")))
    _COMPILED["last_result"] = res
    out = np.zeros((B, S, H), dtype=np.float32)
    for c in range(N_CORES):
        b = c // 4
        q0 = (c % 4) * I_CORE
        out[b, q0:q0 + I_CORE] = np.asarray(
            res.results[c]["out"]).reshape(I_CORE, H)
    return out
